# revision 1
# baseline (speedup 1.0000x reference)
"""Trainium2 Bass kernel for DifferentiableToposAttention.

Math:
  Q = sigmoid(x @ Wq.T + bq); K = sigmoid(x @ Wk.T + bk); V = x @ Wv.T + bv
  truth[q,k] = mean_d min(1 - Q[q,d] + K[k,d], 1) = 1 - (1/D) sum_d relu(Q-K)
  sum_d relu(Q[q,d]-K[k,d]) = sum_d max(Q[q,d],K[k,d]) - sum_d K[k,d]
  logit[q,k] = 10*truth = (10 + (10/D)*sumK[k]) - (10/D) * sum_d max(Q,K)
  masked (k>q) positions get logit 0 exactly (-> softmax weight exp(0)=1),
  matching the reference which fills masked scores with 0.0 before softmax.
  out[q,:] = sum_k softmax(logit)[q,k] * V[k,:]

Sharding: 8 cores, one SPMD program; core c handles batch c//4.  Its 256
queries are eight 32-query sub-tiles in two 128-row groups with
compile-time descending key widths (A: 512/256/128/384, B: 1024/896/768/
640); the host assigns which sub-tile fills each width slot (l = c%4), so
shapes are uniform across cores while causal-mask work is skipped.  Keys
beyond group A's 512 window are all masked there and contribute the
analytic suffix-sum of V with weight exp(0)=1.

Per-core pipeline (layout: d=128 on partitions):
  - xT/KT/QT via PE transposes + matmuls, sigmoid on ACT (bias per-partition)
  - M_q[d,k] = max(KT[d,k], Q[d,q]) fp16 on DVE (4x mode) for rows 0..95,
    relu(Q-K) on the scalar engine for rows 96..127 (engine balance)
  - score row = ones-reduce over partitions via PE matmul; each query's row
    is scattered into PSUM partition q using a sliced stationary that has a
    single all-ones column, writing a 32-partition strip (PSUM-accumulated).
  - logits = (score * -10/128) + Cb on DVE, causal mask multiply,
    exp + row-sum on ACT (accum_out), reciprocal on DVE,
  - AV: transpose exp-weights per 128-block on PE, matmul with V natural,
    add suffix-V (tile A), scale by 1/den, DMA out.
"""

import sys

for _p in ("/opt/trn_rl_repo",):
    if _p not in sys.path:
        sys.path.insert(0, _p)

import numpy as np

import concourse.bass as bass
import concourse.mybir as mybir
import concourse.tile as tile
from concourse import bacc
from concourse.bass import ts
from concourse.masks import make_identity
from concourse.bass_utils import run_bass_kernel_spmd

F32 = mybir.dt.float32
BF16 = mybir.dt.bfloat16
FP16 = mybir.dt.float16
AF = mybir.ActivationFunctionType
ALU = mybir.AluOpType

B, S, D = 2, 1024, 128
NCORES = 8
NQT = S // 128  # 8 query tiles per batch


def _build_program(masked: bool) -> bass.Bass:
    WA = 512 if masked else 1024  # key window width for qtile A
    WB = 1024
    nc = bacc.Bacc()

    xb_d = nc.declare_dram_parameter("xb", [S, D], FP16, isOutput=False)
    xq_d = nc.declare_dram_parameter("xq", [256, D], FP16, isOutput=False)
    wqt_d = nc.declare_dram_parameter("wqt", [D, D], FP16, isOutput=False)
    wkt_d = nc.declare_dram_parameter("wkt", [D, D], FP16, isOutput=False)
    wvt_d = nc.declare_dram_parameter("wvt", [D, D], FP16, isOutput=False)
    bq_d = nc.declare_dram_parameter("bq", [D, 1], F32, isOutput=False)
    bk_d = nc.declare_dram_parameter("bk", [D, 1], F32, isOutput=False)
    bvb_d = nc.declare_dram_parameter("bvb", [D, 4 * D], F32, isOutput=False)
    qrowa_d = nc.declare_dram_parameter("qrowa", [D, 1], F32, isOutput=False)
    qrowb_d = nc.declare_dram_parameter("qrowb", [D, 1], F32, isOutput=False)
    out_d = nc.declare_dram_parameter("out", [256, D], F32, isOutput=True)

    with tile.TileContext(nc) as tc:
        with tc.tile_pool(name="singles", bufs=1) as singles:
            # ---- persistent SBUF tensors ----
            identity_bf = singles.tile([128, 128], FP16)
            make_identity(nc, identity_bf[:])
            ones_col = singles.tile([128, 1], F32)
            nc.vector.memset(ones_col[:], 1.0)
            ones_row = singles.tile([1, 128], F32)
            nc.vector.memset(ones_row[:], 1.0)
            ones_col_bf = singles.tile([128, 1], FP16)
            nc.vector.memset(ones_col_bf[:], 1.0)
            # E64: zeros except column 32 all ones. E64[:, 32-r:64-r] is a
            # [128,32] stationary whose only ones-column is local index r.
            e64 = singles.tile([128, 64], FP16)
            nc.vector.memset(e64[:], 0.0)
            nc.vector.memset(e64[:, 32:33], 1.0)
            e64n = singles.tile([128, 64], FP16)   # -1 column: negated reduce
            nc.vector.memset(e64n[:], 0.0)
            nc.vector.memset(e64n[:, 32:33], -1.0)

            xT = singles.tile([128, S], FP16)       # x^T, batch
            xqT = singles.tile([128, 256], FP16)    # x^T, this core's 256 queries
            QT = singles.tile([128, 256], F32)     # Q^T  [d, q]
            KTb = singles.tile([128, S], FP16)     # K^T in fp16 [d, k]
            Vn = singles.tile([128, NQT, 128], FP16)  # V natural [k(128), blk, e]
            Cb = singles.tile([128, S], F32)       # 10 + (10/D)*sumK[k], bcast
            svb = singles.tile([128, 128], F32)    # suffix-V bcast over q rows
            c_row = singles.tile([1, S], F32)
            sv_row = singles.tile([1, 128], F32)
            iota_r = singles.tile([128, S], F32)
            maskA = singles.tile([128, WA], F32)
            maskB = singles.tile([128, WB], F32)

            wq_sb = singles.tile([128, 128], FP16)
            wk_sb = singles.tile([128, 128], FP16)
            wv_sb = singles.tile([128, 128], FP16)
            bq_sb = singles.tile([128, 1], F32)
            bk_sb = singles.tile([128, 1], F32)
            bvb_sb = singles.tile([128, 4 * 128], F32)
            qra_sb = singles.tile([128, 1], F32)
            exp_warm = singles.tile([128, 1], F32)
            qrb_sb = singles.tile([128, 1], F32)

            nc.gpsimd.dma_start(out=wk_sb[:], in_=wkt_d[:, :])
            nc.gpsimd.dma_start(out=bk_sb[:], in_=bk_d[:, :])
            nc.gpsimd.dma_start(out=wq_sb[:], in_=wqt_d[:, :])
            nc.gpsimd.dma_start(out=bq_sb[:], in_=bq_d[:, :])
            nc.gpsimd.dma_start(out=wv_sb[:], in_=wvt_d[:, :])
            nc.gpsimd.dma_start(out=bvb_sb[:], in_=bvb_d[:, :])
            nc.gpsimd.dma_start(out=qra_sb[:], in_=qrowa_d[:, :])
            nc.gpsimd.dma_start(out=qrb_sb[:], in_=qrowb_d[:, :])

            # causal masks: mask[p, k] = 1.0 iff k <= qrow[p]
            # (iota early on gpsimd; the is_le ops are emitted inside the
            # score loop region so they don't block the first max ops)
            nc.gpsimd.iota(
                iota_r[:], pattern=[[1, S]], base=0, channel_multiplier=0,
                allow_small_or_imprecise_dtypes=True,
            )

            # ---- phase A: transposes + projections ----
            with (
                tc.tile_pool(name="ld", bufs=3) as ld,
                tc.tile_pool(name="ptr", bufs=2, space="PSUM") as ptr,
                tc.tile_pool(name="pp2", bufs=2, space="PSUM") as pp2,
                tc.tile_pool(name="pvv", bufs=2, space="PSUM") as pvv,
                tc.tile_pool(name="prow", bufs=2, space="PSUM") as prow,
            ):
                xbig = ld.tile([128, NQT, 128], FP16, tag="xbig")
                nc.sync.dma_start(
                    out=xbig[:],
                    in_=xb_d.rearrange("(t p) d -> p t d", p=128))
                xqbig = ld.tile([128, 2, 128], FP16, tag="xqbig")
                nc.sync.dma_start(
                    out=xqbig[:],
                    in_=xq_d.rearrange("(t p) d -> p t d", p=128))
                for t in range(NQT):
                    ps = ptr.tile([128, 128], FP16, tag="tr")
                    nc.tensor.transpose(ps[:], xbig[:, t, :], identity_bf[:])
                    nc.vector.tensor_copy(xT[:, ts(t, 128)], ps[:])
                for t in range(2):
                    ps = ptr.tile([128, 128], FP16, tag="tr")
                    nc.tensor.transpose(ps[:], xqbig[:, t, :], identity_bf[:])
                    nc.vector.tensor_copy(xqT[:, ts(t, 128)], ps[:])

                # K^T = (Wk^T)^T @ x^T ; sigmoid(+bk)
                for ch in range(2):
                    psk = pp2.tile([128, 512], F32, tag="proj")
                    nc.tensor.matmul(psk[:], wk_sb[:], xT[:, ts(ch, 512)])
                    nc.scalar.activation(
                        KTb[:, ts(ch, 512)], psk[:], AF.Sigmoid,
                        bias=bk_sb[:], scale=1.0)
                # Q^T for the 256 local queries
                psq = pp2.tile([128, 512], F32, tag="proj")
                nc.tensor.matmul(psq[:, 0:256], wq_sb[:], xqT[:])
                nc.scalar.activation(
                    QT[:], psq[:, 0:256], AF.Sigmoid, bias=bq_sb[:], scale=1.0)
                # preload the exp table set now (after the sigmoids)
                nc.scalar.activation(exp_warm[:], QT[:, 0:1], AF.Exp)


                # sumK row -> Cb = 10 + (10/D) * sumK  broadcast to 128 rows
                for ch in range(2):
                    pss = prow.tile([1, 512], F32, tag="rows")
                    nc.tensor.matmul(pss[:], ones_col_bf[:], KTb[:, ts(ch, 512)])
                    nc.scalar.activation(
                        c_row[:, ts(ch, 512)], pss[:], AF.Copy,
                        bias=10.0, scale=10.0 / D)
                for ch in range(2):
                    psb = pp2.tile([128, 512], F32, tag="proj")
                    nc.tensor.matmul(psb[:], ones_row[:], c_row[:, ts(ch, 512)])
                    nc.scalar.copy(Cb[:, ts(ch, 512)], psb[:])

                # V natural blocks: V[s,e] = x[s,:] @ Wv^T ; + bv (broadcast)
                for half in range(2):
                    psv = pvv.tile([128, 4, 128], F32, tag="vv")
                    for t4 in range(4):
                        t = half * 4 + t4
                        nc.tensor.matmul(psv[:, t4, :], xT[:, ts(t, 128)], wv_sb[:])
                    nc.vector.tensor_add(
                        Vn[:, ts(half, 4), :], psv[:], bvb_sb[:])
                # suffix-V (only needed when masked; zeros otherwise)
                if masked:
                    pssv = prow.tile([1, 512], F32, tag="rows")
                    for t in range(4, NQT):
                        nc.tensor.matmul(
                            pssv[:, 0:128], ones_col_bf[:], Vn[:, t, :],
                            start=(t == 4), stop=(t == NQT - 1))
                    nc.scalar.copy(sv_row[:], pssv[:, 0:128])
                    psvb = prow.tile([128, 128], F32, tag="rows")
                    nc.tensor.matmul(psvb[:], ones_row[:], sv_row[:])
                    nc.scalar.copy(svb[:], psvb[:])


            # ---- phase B: scores + softmax + AV per query tile ----
            with (
                tc.tile_pool(name="mp", bufs=10) as mp,
                tc.tile_pool(name="psc", bufs=2, space="PSUM") as psc,
                tc.tile_pool(name="po", bufs=2, space="PSUM") as po,
                tc.tile_pool(name="pw", bufs=2, space="PSUM") as pw,
                tc.tile_pool(name="sml", bufs=4) as sml,
                tc.tile_pool(name="lg", bufs=3) as lg,
                tc.tile_pool(name="wts", bufs=4) as wtsp,
                tc.tile_pool(name="ob", bufs=2) as ob,
            ):
                for W, qoff, msk, tail, is_a in [
                    (WA, 0, maskA, float(S - WA), True),
                    (WB, 128, maskB, 0.0, False),
                ]:
                    sc = psc.tile([128, 1024], F32, tag="sc")
                    # max-pass + ones-reduce row scatter; r-major order so
                    # consecutive matmuls hit different PE column strips.
                    # With the causal mask, strip s only needs the first
                    # W - 128*s keys (descending sub-tile slots); row r==0
                    # computes/writes the full group width so the strip's
                    # PSUM is fully initialized (start=True) and garbage
                    # beyond a row's slot width is finite (mask zeroes it).
                    for r in range(32):
                        for strip in range(4):
                            q = strip * 32 + r
                            # strip->slot width; strip 3 runs on ACT, so it
                            # gets the 384-wide A slot for engine balance
                            if not masked:
                                ws = W
                            elif is_a:
                                ws = (384, 256, 128, 512)[strip]
                            else:
                                ws = (1024, 768, 640, 896)[strip]
                            wop = W if r == 0 else ws
                            m = mp.tile([128, 1024], FP16, tag="m")
                            qcol = QT[:, qoff + q:qoff + q + 1]
                            e_mat = e64
                            if q < 96:
                                nc.vector.tensor_scalar(
                                    m[:, 0:wop], KTb[:, 0:wop], qcol, None,
                                    ALU.max)
                            else:
                                # relu form: m = relu(Q - K); logit uses the
                                # constant 10.0 instead of Cb for these rows
                                nc.scalar.activation(
                                    m[:, 0:wop], KTb[:, 0:wop], AF.Relu,
                                    bias=qcol, scale=-1.0)
                            nch = (W if r == 0 else min(W, ws + 511)) // 512
                            for ch in range(max(1, nch)):
                                ce = W if r == 0 else ws
                                n = min(512, ce - 512 * ch)
                                nc.tensor.matmul(
                                    sc[ts(strip, 32), 512 * ch:512 * ch + n],
                                    e_mat[:, 32 - r:64 - r],
                                    m[:, 512 * ch:512 * ch + n],
                                    start=(r == 0), stop=(r == 31),
                                    skip_group_check=True,
                                    tile_position=(0, strip * 32),
                                )
                    if is_a:
                        nc.vector.tensor_scalar(
                            maskA[:], iota_r[:, 0:WA], qra_sb[:], None,
                            ALU.is_le)
                        blo = 512 if masked else 0
                        nc.vector.tensor_scalar(
                            maskB[:, blo:WB], iota_r[:, blo:WB], qrb_sb[:],
                            None, ALU.is_le)
                    # logits
                    L = lg.tile([128, 1024], F32, tag="L")
                    fs = 96
                    # column-chunked so the first half of L is complete (and
                    # exp can start) while the second half still computes;
                    # for the wide group the mask only touches cols >= 512
                    for lo, hi in ([(0, 512), (512, W)] if W > 512
                                   else [(0, W)]):
                        nc.vector.scalar_tensor_tensor(
                            out=L[0:fs, lo:hi], in0=sc[0:fs, lo:hi],
                            scalar=-10.0 / D, in1=Cb[0:fs, lo:hi],
                            op0=ALU.mult, op1=ALU.add)
                        nc.vector.tensor_scalar(
                            L[fs:128, lo:hi], sc[fs:128, lo:hi], -10.0 / D,
                            10.0, ALU.mult, ALU.add)
                        mlo = max(lo, 512 if (not is_a and masked) else 0)
                        if mlo < hi:
                            nc.vector.tensor_mul(
                                L[:, mlo:hi], L[:, mlo:hi], msk[:, mlo:hi])
                    # exp + rowsum
                    E = lg.tile([128, 1024], FP16, tag="E")
                    rs = sml.tile([128, 1], F32, tag="rs")
                    den = sml.tile([128, 1], F32, tag="den")
                    if W > 512:
                        rs2 = sml.tile([128, 1], F32, tag="rs2")
                        nc.scalar.activation(
                            E[:, 0:512], L[:, 0:512], AF.Exp, accum_out=rs2[:])
                        nc.scalar.activation(
                            E[:, 512:W], L[:, 512:W], AF.Exp, accum_out=rs[:])
                        nc.vector.tensor_add(rs[:], rs[:], rs2[:])
                    else:
                        nc.scalar.activation(
                            E[:, 0:W], L[:, 0:W], AF.Exp, accum_out=rs[:])
                    nc.vector.tensor_scalar(den[:], rs[:], tail, None, ALU.add)
                    rcp = sml.tile([128, 1], F32, tag="rcp")
                    nc.vector.reciprocal(rcp[:], den[:])
                    # AV
                    o = po.tile([128, 128], F32, tag="o")
                    nblk = W // 128
                    for t in range(nblk):
                        pwt = pw.tile([128, 128], FP16, tag="wt")
                        nc.tensor.transpose(pwt[:], E[:, ts(t, 128)], identity_bf[:])
                        wtile = wtsp.tile([128, 128], FP16, tag="wts")
                        if t % 2 == 0:
                            nc.scalar.copy(wtile[:], pwt[:])
                        else:
                            nc.vector.tensor_copy(wtile[:], pwt[:])
                        nc.tensor.matmul(
                            o[:], wtile[:], Vn[:, t, :],
                            start=(t == 0), stop=(t == nblk - 1))
                    ores = ob.tile([128, 128], F32, tag="ores")
                    if masked and is_a:
                        nc.vector.tensor_add(ores[:], o[:], svb[:])
                        nc.vector.tensor_scalar(
                            ores[:], ores[:], rcp[:], None, ALU.mult)
                    else:
                        nc.vector.tensor_scalar(
                            ores[:], o[:], rcp[:], None, ALU.mult)
                    nc.sync.dma_start(out=out_d[ts(0 if is_a else 1, 128), :], in_=ores[:])

    nc.finalize()
    return nc


_PROG_CACHE: dict[bool, bass.Bass] = {}


def _get_program(masked: bool) -> bass.Bass:
    if masked not in _PROG_CACHE:
        _PROG_CACHE[masked] = _build_program(masked)
    return _PROG_CACHE[masked]


def _core_query_rows(masked: bool, l: int) -> np.ndarray:
    """Global query indices (within the core's batch) for the 256 output
    rows, in on-device row order: group A rows 0..127, group B 128..255.

    Masked: descending width slots; strip s of group A handles 32-query
    sub-tile m = 4*(3-s)+l, group B m = 4*(7-s)+l  (m = q//32).
    Unmasked: contiguous query tiles l and 7-l.
    """
    rows = np.empty(256, dtype=np.int64)
    if masked:
        for s, wslot in enumerate((3, 2, 1, 4)):
            m = 4 * (wslot - 1) + l
            rows[32 * s:32 * s + 32] = 32 * m + np.arange(32)
        for s, wslot in enumerate((8, 6, 5, 7)):
            m = 4 * (wslot - 1) + l
            rows[128 + 32 * s:128 + 32 * s + 32] = 32 * m + np.arange(32)
    else:
        rows[0:128] = 128 * l + np.arange(128)
        rows[128:256] = 128 * (7 - l) + np.arange(128)
    return rows


def build_in_maps(x, Wq, bq, Wk, bk, Wv, bv, masked):
    wqt = np.ascontiguousarray(Wq.T.astype(np.float16))
    wkt = np.ascontiguousarray(Wk.T.astype(np.float16))
    wvt = np.ascontiguousarray(Wv.T.astype(np.float16))
    bq2 = np.ascontiguousarray(bq.reshape(D, 1).astype(np.float32))
    bk2 = np.ascontiguousarray(bk.reshape(D, 1).astype(np.float32))
    bvb = np.ascontiguousarray(
        np.tile(bv.reshape(1, D).astype(np.float32), (D, 4)))
    in_maps = []
    for c in range(NCORES):
        b, l = divmod(c, 4)
        rows = _core_query_rows(masked, l)
        xb = np.ascontiguousarray(x[b].astype(np.float16))
        xq = np.ascontiguousarray(xb[rows])
        if masked:
            qrow = rows.astype(np.float32)
        else:
            qrow = np.full(256, 1e9, dtype=np.float32)
        in_maps.append({
            "xb": xb, "xq": xq, "wqt": wqt, "wkt": wkt, "wvt": wvt,
            "bq": bq2, "bk": bk2, "bvb": bvb,
            "qrowa": np.ascontiguousarray(qrow[0:128].reshape(D, 1)),
            "qrowb": np.ascontiguousarray(qrow[128:256].reshape(D, 1)),
        })
    return in_maps


def assemble_out(results, masked):
    out = np.empty((B, S, D), dtype=np.float32)
    for c in range(NCORES):
        b, l = divmod(c, 4)
        rows = _core_query_rows(masked, l)
        out[b, rows] = results[c]["out"]
    return out


def kernel(x, Wq, bq, Wk, bk, Wv, bv, apply_causal_mask):
    x = np.ascontiguousarray(np.asarray(x, dtype=np.float32))
    Wq = np.asarray(Wq, dtype=np.float32)
    Wk = np.asarray(Wk, dtype=np.float32)
    Wv = np.asarray(Wv, dtype=np.float32)
    bq = np.asarray(bq, dtype=np.float32)
    bk = np.asarray(bk, dtype=np.float32)
    bv = np.asarray(bv, dtype=np.float32)
    masked = bool(int(np.asarray(apply_causal_mask)))

    nc = _get_program(masked)
    in_maps = build_in_maps(x, Wq, bq, Wk, bk, Wv, bv, masked)
    res = run_bass_kernel_spmd(nc, in_maps, list(range(NCORES))).results
    return assemble_out(res, masked)



# revision 3
# speedup vs baseline: 2.3064x; 2.3064x over previous
"""Trainium2 Bass kernel for DifferentiableToposAttention.

Math:
  Q = sigmoid(x @ Wq.T + bq); K = sigmoid(x @ Wk.T + bk); V = x @ Wv.T + bv
  truth[q,k] = 1 - (1/D) sum_d relu(Q[q,d]-K[k,d]);  logit = 10*truth
  masked (k>q) positions get logit 0 exactly (softmax weight exp(0)=1).

Algorithmic core: piecewise-linear feature factorization.  With knots
t_p = p/T (p=0..T, h=1/T) and hat functions phi_p (interp in the K
variable is exact between knots; only the cell containing Q-K's kink
carries O(h^2) error):

  relu(a-b) ~= sum_p phi_p(a) * relu(t_p - b)
  phi_p(a)  = -T * vt_p(a),  vt_p(a) = min(|a - t_p|, h) - h
  relu(t_p-b) = -(min(b, t_p) - t_p) = -m_p(b)

  sum_d relu(Q-K) ~= T * sum_{d,p} vt_p(Q[q,d]) * m_p(K[k,d])  =: T * SC[q,k]

so the whole pairwise nonlinearity becomes ONE dense matmul with
contraction dim D*(T+1), run at 128x128 MACs/cycle on the PE instead of
the 128/cycle of a partition reduce.  logit = 10 - (10T/D)*SC.

Masking uses Z = (SC - D/T) * M1 (M1 host-supplied 0/1), so that
E = exp(-10T/D * Z) gives exp(logit) unmasked and exp(0)=1 masked, with
no bias corrections at all.

Sharding: 8 cores; core c = (b, l) = (c//4, c%4) handles batch b, query
tiles l (keys 0..511 computed) and 4+l (keys 0..1023).  Shapes are
identical across cores (SPMD); causality is entirely in the M1 mask
data.  Keys >= 512 for tile A are all masked: weight-1 contributions
come from an all-ones stationary over V blocks 4..7 plus a +512
denominator constant.
"""

import sys

for _p in ("/opt/trn_rl_repo",):
    if _p not in sys.path:
        sys.path.insert(0, _p)

import numpy as np

import concourse.bass as bass
import concourse.mybir as mybir
import concourse.tile as tile
from concourse import bacc
from concourse.bass import ts
from concourse.masks import make_identity
from concourse.bass_utils import run_bass_kernel_spmd

F32 = mybir.dt.float32
FP16 = mybir.dt.float16
AF = mybir.ActivationFunctionType
ALU = mybir.AluOpType

B, S, D = 2, 1024, 128
NCORES = 8
T = 8                    # knot count (h = 1/T); P = T+1 features per d
P = T + 1
H = 1.0 / T
NEG_DT = -float(D) / T   # Z = (SC + NEG_DT) * M1
EXP_SCALE = -10.0 * T / D


def _build_program(masked: bool) -> bass.Bass:
    WA = 512 if masked else 1024   # computed key width, query tile A (= tile l)
    WB = 1024                      # query tile B (= tile 4+l)
    nc = bacc.Bacc()

    xbt_d = nc.declare_dram_parameter("xbt", [D, S], FP16, isOutput=False)
    xqt_d = nc.declare_dram_parameter("xqt", [D, 256], FP16, isOutput=False)
    wqt_d = nc.declare_dram_parameter("wqt", [D, D], FP16, isOutput=False)
    wkt_d = nc.declare_dram_parameter("wkt", [D, D], FP16, isOutput=False)
    wvt_d = nc.declare_dram_parameter("wvt", [D, D], FP16, isOutput=False)
    bq_d = nc.declare_dram_parameter("bq", [D, 1], F32, isOutput=False)
    bk_d = nc.declare_dram_parameter("bk", [D, 1], F32, isOutput=False)
    bvb_d = nc.declare_dram_parameter("bvb", [D, D], F32, isOutput=False)
    m1a_d = nc.declare_dram_parameter("m1a", [D, WA], FP16, isOutput=False)
    m1b_d = nc.declare_dram_parameter("m1b", [D, WB], FP16, isOutput=False)
    out_d = nc.declare_dram_parameter("out", [256, D], F32, isOutput=True)

    with tile.TileContext(nc) as tc:
        with tc.tile_pool(name="singles", bufs=1) as singles:
            identity = singles.tile([128, 128], FP16)
            make_identity(nc, identity[:])
            ones128 = singles.tile([128, 128], FP16)
            nc.vector.memset(ones128[:], 1.0)

            wq_sb = singles.tile([128, 128], FP16)
            wk_sb = singles.tile([128, 128], FP16)
            wv_sb = singles.tile([128, 128], FP16)
            bq_sb = singles.tile([128, 1], F32)
            bk_sb = singles.tile([128, 1], F32)
            bvb_sb = singles.tile([128, 128], F32)
            xbt_sb = singles.tile([128, S], FP16)
            xqt_sb = singles.tile([128, 256], FP16)
            m1a_sb = singles.tile([128, WA], FP16)
            m1b_sb = singles.tile([128, WB], FP16)

            KTb = singles.tile([128, S], FP16)     # sigmoid K^T  [d, k]
            QTb = singles.tile([128, 256], FP16)   # sigmoid Q^T  [d, q]
            Vn = singles.tile([128, 8, 128], FP16)  # V (no bv)  [k, blk, e]
            mp = singles.tile([128, P, S], FP16)    # moving feats min(K,t)-t
            vt = singles.tile([128, P, 256], FP16)  # stationary feats
            EA = singles.tile([128, WA], FP16)
            EB = singles.tile([128, WB], FP16)
            exp_warm = singles.tile([128, 1], F32)

            nc.gpsimd.dma_start(out=wk_sb[:], in_=wkt_d[:, :])
            nc.gpsimd.dma_start(out=bk_sb[:], in_=bk_d[:, :])
            nc.gpsimd.dma_start(out=wq_sb[:], in_=wqt_d[:, :])
            nc.gpsimd.dma_start(out=bq_sb[:], in_=bq_d[:, :])
            nc.gpsimd.dma_start(out=wv_sb[:], in_=wvt_d[:, :])
            nc.gpsimd.dma_start(out=bvb_sb[:], in_=bvb_d[:, :])
            nc.sync.dma_start(out=xbt_sb[:], in_=xbt_d[:, :])
            nc.sync.dma_start(out=xqt_sb[:], in_=xqt_d[:, :])
            nc.sync.dma_start(out=m1a_sb[:], in_=m1a_d[:, :])
            nc.sync.dma_start(out=m1b_sb[:], in_=m1b_d[:, :])

            # ---- phase 1: projections ----
            with (
                tc.tile_pool(name="pproj", bufs=2, space="PSUM") as pproj,
                tc.tile_pool(name="pvv", bufs=2, space="PSUM") as pvv,
            ):
                psk = pproj.tile([128, S], F32, tag="proj")
                nc.tensor.matmul(psk[:, 0:512], wk_sb[:], xbt_sb[:, 0:512])
                nc.tensor.matmul(psk[:, 512:1024], wk_sb[:], xbt_sb[:, 512:1024])
                nc.scalar.activation(
                    KTb[:], psk[:], AF.Sigmoid, bias=bk_sb[:], scale=1.0)
                psq = pproj.tile([128, 256], F32, tag="projq")
                nc.tensor.matmul(psq[:], wq_sb[:], xqt_sb[:])
                nc.scalar.activation(
                    QTb[:], psq[:], AF.Sigmoid, bias=bq_sb[:], scale=1.0)
                # preload exp table set right after the sigmoids
                nc.scalar.activation(exp_warm[:], QTb[:, 0:1], AF.Exp)

                for half in range(2):
                    psv = pvv.tile([128, 4, 128], F32, tag="vv")
                    for j4 in range(4):
                        j = half * 4 + j4
                        nc.tensor.matmul(
                            psv[:, j4, :], xbt_sb[:, ts(j, 128)], wv_sb[:])
                    if half == 0:
                        nc.vector.tensor_copy(Vn[:, 0:4, :], psv[:])
                    else:
                        nc.scalar.copy(Vn[:, 4:8, :], psv[:])

            # ---- phase 2: features + score matmuls ----
            with (
                tc.tile_pool(name="utmp", bufs=3) as utp,
                tc.tile_pool(name="psc", bufs=1, space="PSUM") as psc,
            ):
                QN = utp.tile([128, 256], FP16, tag="qn")
                nc.vector.tensor_scalar(QN[:], QTb[:], -1.0, None, ALU.mult)
                scA = psc.tile([128, WA], F32, tag="scA")
                scB = psc.tile([128, WB], F32, tag="scB")
                for p in range(P):
                    t_p = p * H
                    # moving: m_p = min(K, t_p) - t_p   [d, k]
                    nc.vector.tensor_scalar(
                        mp[:, p, :], KTb[:], t_p, t_p, ALU.min, ALU.subtract)
                    # stationary: vt_p = min(|Q - t_p|, h) - h   [d, q]
                    # via uh = max(-Q + 2t_p, Q) = |Q - t_p| + t_p
                    u = utp.tile([128, 256], FP16, tag="u")
                    nc.vector.scalar_tensor_tensor(
                        out=u[:], in0=QN[:], scalar=2.0 * t_p, in1=QTb[:],
                        op0=ALU.add, op1=ALU.max)
                    nc.vector.tensor_scalar(
                        vt[:, p, :], u[:], t_p + H, t_p + H,
                        ALU.min, ALU.subtract)
                for p in range(P):
                    st, sp = (p == 0), (p == P - 1)
                    nc.tensor.matmul(
                        scA[:], vt[:, p, 0:128], mp[:, p, 0:WA],
                        start=st, stop=sp)
                    nc.tensor.matmul(
                        scB[:, 0:512], vt[:, p, 128:256], mp[:, p, 0:512],
                        start=st, stop=sp)
                    nc.tensor.matmul(
                        scB[:, 512:1024], vt[:, p, 128:256], mp[:, p, 512:1024],
                        start=st, stop=sp)

                # ---- phase 3: mask + exp + denominators ----
                with tc.tile_pool(name="sml", bufs=1) as sml:
                    ZA = utp.tile([128, WA], FP16, tag="ZA")
                    ZB = utp.tile([128, WB], FP16, tag="ZB")
                    rsA = sml.tile([128, 1], F32)
                    rsB = sml.tile([128, 1], F32)
                    denA = sml.tile([128, 1], F32)
                    rcpA = sml.tile([128, 1], F32)
                    rcpB = sml.tile([128, 1], F32)
                    nc.vector.scalar_tensor_tensor(
                        out=ZA[:], in0=scA[:], scalar=NEG_DT, in1=m1a_sb[:],
                        op0=ALU.add, op1=ALU.mult)
                    nc.scalar.activation(
                        EA[:], ZA[:], AF.Exp, scale=EXP_SCALE, accum_out=rsA[:])
                    nc.vector.scalar_tensor_tensor(
                        out=ZB[:], in0=scB[:], scalar=NEG_DT, in1=m1b_sb[:],
                        op0=ALU.add, op1=ALU.mult)
                    nc.scalar.activation(
                        EB[:], ZB[:], AF.Exp, scale=EXP_SCALE, accum_out=rsB[:])
                    if masked:
                        nc.vector.tensor_scalar(
                            denA[:], rsA[:], float(S - WA), None, ALU.add)
                        nc.vector.reciprocal(rcpA[:], denA[:])
                    else:
                        nc.vector.reciprocal(rcpA[:], rsA[:])
                    nc.vector.reciprocal(rcpB[:], rsB[:])

                    # ---- phase 4: AV ----
                    with (
                        tc.tile_pool(name="pw", bufs=2, space="PSUM") as pw,
                        tc.tile_pool(name="po", bufs=2, space="PSUM") as po,
                        tc.tile_pool(name="wts", bufs=3) as wtsp,
                        tc.tile_pool(name="ob", bufs=2) as ob,
                    ):
                        for is_a, E, W, rcp in (
                            (True, EA, WA, rcpA), (False, EB, WB, rcpB)):
                            o = po.tile([128, 128], F32, tag="o")
                            nblk = W // 128
                            last_mm = 8 if (is_a and masked) else nblk
                            for j in range(nblk):
                                pwt = pw.tile([128, 128], FP16, tag="wt")
                                nc.tensor.transpose(
                                    pwt[:], E[:, ts(j, 128)], identity[:])
                                wtile = wtsp.tile([128, 128], FP16, tag="wts")
                                if j % 2 == 0:
                                    nc.scalar.copy(wtile[:], pwt[:])
                                else:
                                    nc.vector.tensor_copy(wtile[:], pwt[:])
                                nc.tensor.matmul(
                                    o[:], wtile[:], Vn[:, j, :],
                                    start=(j == 0), stop=(j == last_mm - 1))
                            if is_a and masked:
                                # masked tail keys 512..1023: weight-1
                                for j in range(4, 8):
                                    nc.tensor.matmul(
                                        o[:], ones128[:], Vn[:, j, :],
                                        start=False, stop=(j == 7))
                            ores = ob.tile([128, 128], F32, tag="ores")
                            nc.vector.scalar_tensor_tensor(
                                out=ores[:], in0=o[:], scalar=rcp[:],
                                in1=bvb_sb[:], op0=ALU.mult, op1=ALU.add)
                            nc.sync.dma_start(
                                out=out_d[ts(0 if is_a else 1, 128), :],
                                in_=ores[:])

    nc.finalize()
    return nc


_PROG_CACHE: dict[bool, bass.Bass] = {}


def _get_program(masked: bool) -> bass.Bass:
    if masked not in _PROG_CACHE:
        _PROG_CACHE[masked] = _build_program(masked)
    return _PROG_CACHE[masked]


def build_in_maps(x, Wq, bq, Wk, bk, Wv, bv, masked):
    wqt = np.ascontiguousarray(Wq.T.astype(np.float16))
    wkt = np.ascontiguousarray(Wk.T.astype(np.float16))
    wvt = np.ascontiguousarray(Wv.T.astype(np.float16))
    bq2 = np.ascontiguousarray(bq.reshape(D, 1).astype(np.float32))
    bk2 = np.ascontiguousarray(bk.reshape(D, 1).astype(np.float32))
    bvb = np.ascontiguousarray(
        np.tile(bv.reshape(1, D).astype(np.float32), (D, 1)))
    WA = 512 if masked else 1024
    WB = 1024
    kidx = np.arange(S)
    in_maps = []
    for c in range(NCORES):
        b, l = divmod(c, 4)
        xb16 = x[b].astype(np.float16)
        xbt = np.ascontiguousarray(xb16.T)
        rows = np.concatenate(
            [128 * l + np.arange(128), 128 * (4 + l) + np.arange(128)])
        xqt = np.ascontiguousarray(xb16[rows].T)
        if masked:
            qa = (128 * l + np.arange(128))[:, None]
            qb = (128 * (4 + l) + np.arange(128))[:, None]
            m1a = (kidx[None, :WA] <= qa).astype(np.float16)
            m1b = (kidx[None, :WB] <= qb).astype(np.float16)
        else:
            m1a = np.ones((128, WA), np.float16)
            m1b = np.ones((128, WB), np.float16)
        in_maps.append({
            "xbt": xbt, "xqt": xqt, "wqt": wqt, "wkt": wkt, "wvt": wvt,
            "bq": bq2, "bk": bk2, "bvb": bvb,
            "m1a": np.ascontiguousarray(m1a),
            "m1b": np.ascontiguousarray(m1b),
        })
    return in_maps


def assemble_out(results, masked):
    out = np.empty((B, S, D), dtype=np.float32)
    for c in range(NCORES):
        b, l = divmod(c, 4)
        res = results[c]["out"]
        out[b, 128 * l:128 * (l + 1)] = res[0:128]
        out[b, 128 * (4 + l):128 * (5 + l)] = res[128:256]
    return out


def kernel(x, Wq, bq, Wk, bk, Wv, bv, apply_causal_mask):
    x = np.ascontiguousarray(np.asarray(x, dtype=np.float32))
    Wq = np.asarray(Wq, dtype=np.float32)
    Wk = np.asarray(Wk, dtype=np.float32)
    Wv = np.asarray(Wv, dtype=np.float32)
    bq = np.asarray(bq, dtype=np.float32)
    bk = np.asarray(bk, dtype=np.float32)
    bv = np.asarray(bv, dtype=np.float32)
    masked = bool(int(np.asarray(apply_causal_mask)))

    nc = _get_program(masked)
    in_maps = build_in_maps(x, Wq, bq, Wk, bk, Wv, bv, masked)
    res = run_bass_kernel_spmd(nc, in_maps, list(range(NCORES))).results
    return assemble_out(res, masked)


# revision 7
# speedup vs baseline: 2.4380x; 1.0571x over previous
"""Trainium2 Bass kernel for DifferentiableToposAttention.

Math:
  Q = sigmoid(x @ Wq.T + bq); K = sigmoid(x @ Wk.T + bk); V = x @ Wv.T + bv
  truth[q,k] = 1 - (1/D) sum_d relu(Q[q,d]-K[k,d]);  logit = 10*truth
  masked (k>q) positions get logit 0 exactly (softmax weight exp(0)=1).

Algorithmic core: piecewise-linear feature factorization.  With knots
t_p = p/T (p=0..T, h=1/T) and hat functions phi_p (interpolation in the
K variable is exact between knots; only the cell containing the kink of
relu carries O(h^2) error):

  relu(a-b) ~= sum_p phi_p(a) * relu(t_p - b)
  phi_p(a)  = -T * vt_p(a),  vt_p(a) = min(|a - t_p|, h) - h
  relu(t_p-b) = -(min(b, t_p) - t_p) = -m_p(b)

  sum_d relu(Q-K) ~= T * sum_{d,p} vt_p(Q[q,d]) * m_p(K[k,d]) =: T * SC

so the whole pairwise nonlinearity becomes one dense matmul with
contraction dim D*(T+1), run at 128x128 MACs/cycle on the PE instead of
the 128/cycle of a partition reduce.  logit = 10 - (10T/D)*SC.

Masking uses Z = (SC - D/T) * M1 (M1 host-supplied 0/1), so that
E = exp(-10T/D * Z) gives exp(logit) unmasked and exp(0)=1 masked, with
no bias corrections.

Sharding: 8 cores; core c = (b, l) = (c//4, c%4) handles batch b, query
tiles l (keys 0..511 computed) and 4+l (keys 0..1023).  Shapes are
identical across cores (SPMD); causality is entirely in the M1 mask
data.  Keys >= 512 for tile A are all masked: weight-1 contributions
come from an all-ones stationary over V blocks 4..7 plus a +512
denominator constant.

Pipelining: score PSUM is built in 512-wide chunks in order A, B0, B1;
each chunk's Z -> exp -> EtT transposes -> AV matmuls overlap the next
chunk's score matmuls.
"""

import sys

for _p in ("/opt/trn_rl_repo",):
    if _p not in sys.path:
        sys.path.insert(0, _p)

import numpy as np

import concourse.bass as bass
import concourse.mybir as mybir
import concourse.tile as tile
from concourse import bacc
from concourse.bass import ts
from concourse.masks import make_identity
from concourse.bass_utils import run_bass_kernel_spmd

F32 = mybir.dt.float32
FP16 = mybir.dt.float16
AF = mybir.ActivationFunctionType
ALU = mybir.AluOpType

B, S, D = 2, 1024, 128
NCORES = 8
T = 8                    # knot count (h = 1/T); P = T+1 features per d
P = T + 1
H = 1.0 / T
NEG_DT = -float(D) / T   # Z = (SC + NEG_DT) * M1
EXP_SCALE = -10.0 * T / D


def _build_program(masked: bool) -> bass.Bass:
    WA = 512 if masked else 1024   # computed key width, query tile A (tile l)
    WB = 1024                      # query tile B (tile 4+l)
    nc = bacc.Bacc()

    xbt_d = nc.declare_dram_parameter("xbt", [D, S], FP16, isOutput=False)
    xqt_d = nc.declare_dram_parameter("xqt", [D, 256], FP16, isOutput=False)
    # wcat = [Wk.T | Wq.T | Wv.T]  (fused: one DMA descriptor)
    wcat_d = nc.declare_dram_parameter("wcat", [D, 3 * D], FP16, isOutput=False)
    # bcat = [bk | bq | bvb(128 cols)]
    bcat_d = nc.declare_dram_parameter("bcat", [D, 2 + D], F32, isOutput=False)
    m1a_d = nc.declare_dram_parameter("m1a", [D, WA], FP16, isOutput=False)
    m1b_d = nc.declare_dram_parameter("m1b", [D, WB], FP16, isOutput=False)
    out_d = nc.declare_dram_parameter("out", [256, D], F32, isOutput=True)

    with tile.TileContext(nc) as tc:
        with tc.tile_pool(name="singles", bufs=1) as singles:
            wcat_sb = singles.tile([128, 3 * 128], FP16)
            bcat_sb = singles.tile([128, 2 + 128], F32)
            xbt_sb = singles.tile([128, S], FP16)
            xqt_sb = singles.tile([128, 256], FP16)
            m1a_sb = singles.tile([128, WA], FP16)
            m1b_sb = singles.tile([128, WB], FP16)

            # input DMAs first: keep the queues free so weights/x land early
            nc.sync.dma_start(out=xbt_sb[:], in_=xbt_d[:, :])
            nc.gpsimd.dma_start(out=wcat_sb[:], in_=wcat_d[:, :])
            nc.gpsimd.dma_start(out=bcat_sb[:], in_=bcat_d[:, :])
            nc.sync.dma_start(out=xqt_sb[:], in_=xqt_d[:, :])
            nc.sync.dma_start(out=m1a_sb[:], in_=m1a_d[:, :])
            nc.sync.dma_start(out=m1b_sb[:], in_=m1b_d[:, :])

            wk_sb = wcat_sb[:, 0:128]
            wq_sb = wcat_sb[:, 128:256]
            wv_sb = wcat_sb[:, 256:384]
            bk_sb = bcat_sb[:, 0:1]
            bq_sb = bcat_sb[:, 1:2]
            bvb_sb = bcat_sb[:, 2:130]

            identity = singles.tile([128, 128], FP16)
            make_identity(nc, identity[:])
            ones128 = singles.tile([128, 128], FP16)
            nc.vector.memset(ones128[:], 1.0)
            warm = singles.tile([128, 1], F32)
            # pull the sigmoid ACT table load to t~0 (no data deps)
            nc.scalar.activation(warm[:], ones128[:, 0:1], AF.Sigmoid)

            KTb = singles.tile([128, S], FP16)     # sigmoid K^T  [d, k]
            QTb = singles.tile([128, 256], FP16)   # sigmoid Q^T  [d, q]
            Vn = singles.tile([128, 8, 128], FP16)  # V (no bv)  [k, blk, e]
            mp = singles.tile([128, P, S], FP16)    # moving feats min(K,t)-t
            vt = singles.tile([128, P, 256], FP16)  # stationary feats

            # ---- phase 1: projections ----
            with (
                tc.tile_pool(name="pproj", bufs=2, space="PSUM") as pproj,
                tc.tile_pool(name="pvv", bufs=2, space="PSUM") as pvv,
            ):
                psk = pproj.tile([128, S], F32, tag="proj")
                nc.tensor.matmul(psk[:, 0:512], wk_sb, xbt_sb[:, 0:512])
                nc.tensor.matmul(psk[:, 512:1024], wk_sb, xbt_sb[:, 512:1024])
                nc.scalar.activation(
                    KTb[:], psk[:], AF.Sigmoid, bias=bk_sb, scale=1.0)
                psq = pproj.tile([128, 256], F32, tag="projq")
                nc.tensor.matmul(psq[:], wq_sb, xqt_sb[:])
                nc.scalar.activation(
                    QTb[:], psq[:], AF.Sigmoid, bias=bq_sb, scale=1.0)
                # preload the exp table set right after the sigmoids
                nc.scalar.activation(warm[:], QTb[:, 0:1], AF.Exp)

                for half in range(2):
                    psv = pvv.tile([128, 4, 128], F32, tag="vv")
                    for j4 in range(4):
                        j = half * 4 + j4
                        nc.tensor.matmul(
                            psv[:, j4, :], xbt_sb[:, ts(j, 128)], wv_sb)
                    if half == 0:
                        nc.vector.tensor_copy(Vn[:, 0:4, :], psv[:])
                    else:
                        nc.scalar.copy(Vn[:, 4:8, :], psv[:])

            # ---- phase 2+3+4: chunked score -> mask+exp -> AV pipeline ----
            with (
                tc.tile_pool(name="utmp", bufs=4) as utp,
                tc.tile_pool(name="psc", bufs=1, space="PSUM") as psc,
                tc.tile_pool(name="ezt", bufs=1) as ezt,
                tc.tile_pool(name="sml", bufs=1) as sml,
                tc.tile_pool(name="pw", bufs=2, space="PSUM") as pw,
                tc.tile_pool(name="po", bufs=1, space="PSUM") as po,
                tc.tile_pool(name="wts", bufs=3) as wtsp,
                tc.tile_pool(name="ob", bufs=2) as ob,
            ):
                QN = utp.tile([128, 256], FP16, tag="qn")
                nc.vector.tensor_scalar(QN[:], QTb[:], -1.0, None, ALU.mult)
                for p in range(P):
                    t_p = p * H
                    # moving: m_p = min(K, t_p) - t_p   [d, k]
                    nc.vector.tensor_scalar(
                        mp[:, p, :], KTb[:], t_p, t_p, ALU.min, ALU.subtract)
                    # stationary: vt_p = min(|Q-t_p|, h) - h
                    #   = max(min(Q-t_p-h, 0), min(-Q+t_p-h, 0))
                    a = utp.tile([128, 256], FP16, tag="ua")
                    b = utp.tile([128, 256], FP16, tag="ub")
                    nc.vector.tensor_scalar(
                        a[:], QTb[:], t_p + H, 0.0, ALU.subtract, ALU.min)
                    nc.vector.tensor_scalar(
                        b[:], QN[:], t_p - H, 0.0, ALU.add, ALU.min)
                    nc.vector.tensor_max(vt[:, p, :], a[:], b[:])

                scA = psc.tile([128, WA], F32, tag="scA")
                scB = psc.tile([128, WB], F32, tag="scB")
                EA = ezt.tile([128, WA], FP16)
                EB = ezt.tile([128, WB], FP16)
                NCA, NCB = WA // 512, WB // 512
                rsA0 = sml.tile([128, 1], F32)
                rsA1 = sml.tile([128, 1], F32)
                rsB0 = sml.tile([128, 1], F32)
                rsB1 = sml.tile([128, 1], F32)
                rs = {("A", 0): rsA0, ("A", 1): rsA1,
                      ("B", 0): rsB0, ("B", 1): rsB1}
                denA = sml.tile([128, 1], F32)
                denB = sml.tile([128, 1], F32)
                rcpA = sml.tile([128, 1], F32)
                rcpB = sml.tile([128, 1], F32)
                oA = po.tile([128, 128], F32, tag="oA")
                oB = po.tile([128, 128], F32, tag="oB")

                chunks = [("A", ca) for ca in range(NCA)]
                chunks += [("B", cb) for cb in range(NCB)]

                def emit_score(tile_id, ci):
                    sc = scA if tile_id == "A" else scB
                    qlo = 0 if tile_id == "A" else 128
                    for p in range(P):
                        nc.tensor.matmul(
                            sc[:, ts(ci, 512)],
                            vt[:, p, qlo:qlo + 128],
                            mp[:, p, ts(ci, 512)],
                            start=(p == 0), stop=(p == P - 1))

                def emit_tail(tile_id, ci, close):
                    sc, E = (scA, EA) if tile_id == "A" else (scB, EB)
                    m1 = m1a_sb if tile_id == "A" else m1b_sb
                    Z = utp.tile([128, 512], FP16, tag="z")
                    nc.vector.scalar_tensor_tensor(
                        out=Z[:], in0=sc[:, ts(ci, 512)], scalar=NEG_DT,
                        in1=m1[:, ts(ci, 512)], op0=ALU.add, op1=ALU.mult)
                    nc.scalar.activation(
                        E[:, ts(ci, 512)], Z[:], AF.Exp, scale=EXP_SCALE,
                        accum_out=rs[(tile_id, ci)][:])
                    o = oA if tile_id == "A" else oB
                    for j4 in range(4):
                        j = ci * 4 + j4
                        pwt = pw.tile([128, 128], FP16, tag="wt")
                        nc.tensor.transpose(
                            pwt[:], E[:, ts(j, 128)], identity[:])
                        wtile = wtsp.tile([128, 128], FP16, tag="wts")
                        if j % 2 == 0:
                            nc.scalar.copy(wtile[:], pwt[:])
                        else:
                            nc.vector.tensor_copy(wtile[:], pwt[:])
                        nc.tensor.matmul(
                            o[:], wtile[:], Vn[:, j, :],
                            start=(j == 0), stop=(close and j4 == 3))

                def emit_out(tile_id, o, rcp, rows):
                    ores = ob.tile([128, 128], F32, tag="ores")
                    nc.vector.scalar_tensor_tensor(
                        out=ores[:], in0=o[:], scalar=rcp[:],
                        in1=bvb_sb, op0=ALU.mult, op1=ALU.add)
                    nc.sync.dma_start(out=out_d[rows, :], in_=ores[:])

                for (tile_id, ci) in chunks:
                    last = ci == (NCA if tile_id == "A" else NCB) - 1
                    emit_score(tile_id, ci)
                    emit_tail(tile_id, ci, close=(last and not
                                                  (tile_id == "A" and masked)))
                    if tile_id == "A" and last:
                        if masked:
                            # masked tail keys 512..1023: weight-1
                            for j in range(4, 8):
                                nc.tensor.matmul(
                                    oA[:], ones128[:], Vn[:, j, :],
                                    start=False, stop=(j == 7))
                            nc.vector.tensor_scalar(
                                denA[:], rs[("A", 0)][:], float(S - WA),
                                None, ALU.add)
                        else:
                            nc.vector.tensor_add(
                                denA[:], rs[("A", 0)][:], rs[("A", 1)][:])
                        nc.vector.reciprocal(rcpA[:], denA[:])
                        emit_out("A", oA, rcpA, slice(0, 128))
                    if tile_id == "B" and last:
                        nc.vector.tensor_add(
                            denB[:], rs[("B", 0)][:], rs[("B", 1)][:])
                        nc.vector.reciprocal(rcpB[:], denB[:])
                        emit_out("B", oB, rcpB, slice(128, 256))

    nc.finalize()
    return nc


_PROG_CACHE: dict[bool, bass.Bass] = {}


def _get_program(masked: bool) -> bass.Bass:
    if masked not in _PROG_CACHE:
        _PROG_CACHE[masked] = _build_program(masked)
    return _PROG_CACHE[masked]


def build_in_maps(x, Wq, bq, Wk, bk, Wv, bv, masked):
    wcat = np.ascontiguousarray(
        np.concatenate([Wk.T, Wq.T, Wv.T], axis=1).astype(np.float16))
    bcat = np.ascontiguousarray(
        np.concatenate(
            [bk.reshape(D, 1), bq.reshape(D, 1),
             np.tile(bv.reshape(1, D), (D, 1))], axis=1).astype(np.float32))
    WA = 512 if masked else 1024
    WB = 1024
    kidx = np.arange(S)
    in_maps = []
    for c in range(NCORES):
        b, l = divmod(c, 4)
        xb16 = x[b].astype(np.float16)
        xbt = np.ascontiguousarray(xb16.T)
        rows = np.concatenate(
            [128 * l + np.arange(128), 128 * (4 + l) + np.arange(128)])
        xqt = np.ascontiguousarray(xb16[rows].T)
        if masked:
            qa = (128 * l + np.arange(128))[:, None]
            qb = (128 * (4 + l) + np.arange(128))[:, None]
            m1a = (kidx[None, :WA] <= qa).astype(np.float16)
            m1b = (kidx[None, :WB] <= qb).astype(np.float16)
        else:
            m1a = np.ones((128, WA), np.float16)
            m1b = np.ones((128, WB), np.float16)
        in_maps.append({
            "xbt": xbt, "xqt": xqt, "wcat": wcat, "bcat": bcat,
            "m1a": np.ascontiguousarray(m1a),
            "m1b": np.ascontiguousarray(m1b),
        })
    return in_maps


def assemble_out(results, masked):
    out = np.empty((B, S, D), dtype=np.float32)
    for c in range(NCORES):
        b, l = divmod(c, 4)
        res = results[c]["out"]
        out[b, 128 * l:128 * (l + 1)] = res[0:128]
        out[b, 128 * (4 + l):128 * (5 + l)] = res[128:256]
    return out


def kernel(x, Wq, bq, Wk, bk, Wv, bv, apply_causal_mask):
    x = np.ascontiguousarray(np.asarray(x, dtype=np.float32))
    Wq = np.asarray(Wq, dtype=np.float32)
    Wk = np.asarray(Wk, dtype=np.float32)
    Wv = np.asarray(Wv, dtype=np.float32)
    bq = np.asarray(bq, dtype=np.float32)
    bk = np.asarray(bk, dtype=np.float32)
    bv = np.asarray(bv, dtype=np.float32)
    masked = bool(int(np.asarray(apply_causal_mask)))

    nc = _get_program(masked)
    in_maps = build_in_maps(x, Wq, bq, Wk, bk, Wv, bv, masked)
    res = run_bass_kernel_spmd(nc, in_maps, list(range(NCORES))).results
    return assemble_out(res, masked)


# revision 13
# speedup vs baseline: 2.5528x; 1.0471x over previous
"""Trainium2 Bass kernel for DifferentiableToposAttention.

Math:
  Q = sigmoid(x @ Wq.T + bq); K = sigmoid(x @ Wk.T + bk); V = x @ Wv.T + bv
  truth[q,k] = 1 - (1/D) sum_d relu(Q[q,d]-K[k,d]);  logit = 10*truth
  masked (k>q) positions get logit 0 exactly (softmax weight exp(0)=1).

Algorithmic core: piecewise-linear feature factorization.  With knots
t_p = p/T (p=0..T, h=1/T) and hat functions phi_p (interpolation in the
K variable is exact between knots; only the cell containing the kink of
relu carries O(h^2) error):

  relu(a-b) ~= sum_p phi_p(a) * relu(t_p - b)
  phi_p(a)  = -T * vt_p(a),  vt_p(a) = min(|a - t_p|, h) - h
  relu(t_p-b) = -(min(b, t_p) - t_p) = -m_p(b)

  sum_d relu(Q-K) ~= T * sum_{d,p} vt_p(Q[q,d]) * m_p(K[k,d]) =: T * SC

so the whole pairwise nonlinearity becomes one dense matmul with
contraction dim D*(T+1), run at 128x128 MACs/cycle on the PE instead of
the 128/cycle of a partition reduce.  logit = 10 - (10T/D)*SC.

Masking uses Z = (SC - D/T) * M1 (M1 host-supplied 0/1), so that
E = exp(-10T/D * Z) gives exp(logit) unmasked and exp(0)=1 masked, with
no bias corrections.

Sharding: 8 cores; core c = (b, l) = (c//4, c%4) handles batch b, query
tiles l (keys 0..511 computed) and 4+l (keys 0..1023).  Shapes are
identical across cores (SPMD); causality is entirely in the M1 mask
data.  Keys >= 512 for tile A are all masked: weight-1 contributions
come from an all-ones stationary over V blocks 4..7 plus a +512
denominator constant.

Pipelining: score PSUM is built in 512-wide chunks in order A, B0, B1;
each chunk's Z -> exp -> EtT transposes -> AV matmuls overlap the next
chunk's score matmuls.
"""

import sys

for _p in ("/opt/trn_rl_repo",):
    if _p not in sys.path:
        sys.path.insert(0, _p)

import numpy as np

import concourse.bass as bass
import concourse.mybir as mybir
import concourse.tile as tile
from concourse import bacc
from concourse.bass import ts
from concourse.masks import make_identity
from concourse.bass_utils import run_bass_kernel_spmd

F32 = mybir.dt.float32
FP16 = mybir.dt.float16
AF = mybir.ActivationFunctionType
ALU = mybir.AluOpType

B, S, D = 2, 1024, 128
NCORES = 8
T = 8                    # knot count (h = 1/T); P = T+1 features per d
P = T + 1
H = 1.0 / T
NEG_DT = -float(D) / T   # Z = (SC + NEG_DT) * M1
EXP_SCALE = -10.0 * T / D


def _build_program(masked: bool) -> bass.Bass:
    WA = 512 if masked else 1024   # computed key width, query tile A (tile l)
    WB = 1024                      # query tile B (tile 4+l)
    nc = bacc.Bacc()

    xbt_d = nc.declare_dram_parameter("xbt", [D, S], FP16, isOutput=False)
    xqt_d = nc.declare_dram_parameter("xqt", [D, 256], FP16, isOutput=False)
    wk_d = nc.declare_dram_parameter("wk", [D, D], FP16, isOutput=False)
    wq_d = nc.declare_dram_parameter("wq", [D, D], FP16, isOutput=False)
    wv_d = nc.declare_dram_parameter("wv", [D, D], FP16, isOutput=False)
    # bcat = [bk | bq | bvb(128 cols)]
    bcat_d = nc.declare_dram_parameter("bcat", [D, 2 + D], F32, isOutput=False)
    m1a_d = nc.declare_dram_parameter("m1a", [D, WA], FP16, isOutput=False)
    m1b_d = nc.declare_dram_parameter("m1b", [D, WB], FP16, isOutput=False)
    out_d = nc.declare_dram_parameter("out", [256, D], F32, isOutput=True)

    with tile.TileContext(nc) as tc:
        with tc.tile_pool(name="singles", bufs=1) as singles:
            wk_sb = singles.tile([128, 128], FP16)
            wq_sb = singles.tile([128, 128], FP16)
            wv_sb = singles.tile([128, 128], FP16)
            bcat_sb = singles.tile([128, 2 + 128], F32)
            xbt_sb = singles.tile([128, S], FP16)
            xqt_sb = singles.tile([128, 256], FP16)
            m1a_sb = singles.tile([128, WA], FP16)
            m1b_sb = singles.tile([128, WB], FP16)

            identity = singles.tile([128, 128], FP16)
            make_identity(nc, identity[:])
            warm512 = singles.tile([128, 512], FP16)
            nc.vector.memset(warm512[:], 1.0)
            ones128 = singles.tile([128, 128], FP16)
            nc.vector.memset(ones128[:], 1.0)

            # input DMAs spread across engine queues so they complete in
            # parallel; x/weights (needed first) lead
            nc.sync.dma_start(out=xbt_sb[:], in_=xbt_d[:, :])
            nc.scalar.dma_start(out=wq_sb[:], in_=wq_d[:, :])
            nc.scalar.dma_start(out=xqt_sb[:], in_=xqt_d[:, :])
            nc.gpsimd.dma_start(out=wk_sb[:], in_=wk_d[:, :])
            nc.gpsimd.dma_start(out=wv_sb[:], in_=wv_d[:, :])
            nc.gpsimd.dma_start(out=bcat_sb[:], in_=bcat_d[:, :])
            nc.sync.dma_start(out=m1a_sb[:], in_=m1a_d[:, :])
            nc.sync.dma_start(out=m1b_sb[:], in_=m1b_d[:, :])

            bk_sb = bcat_sb[:, 0:1]
            bq_sb = bcat_sb[:, 1:2]
            bvb_sb = bcat_sb[:, 2:130]

            warm = singles.tile([128, 1], F32)
            # pull the sigmoid ACT table load to t~0 (no data deps)
            nc.scalar.activation(warm[:], ones128[:, 0:1], AF.Sigmoid)

            KTb = singles.tile([128, S], FP16)     # sigmoid K^T  [d, k]
            QTb = singles.tile([128, 256], FP16)   # sigmoid Q^T  [d, q]
            Vn = singles.tile([128, 8, 128], FP16)  # V (no bv)  [k, blk, e]
            mp = singles.tile([128, P, S], FP16)    # moving feats min(K,t)-t
            vt = singles.tile([128, P, 256], FP16)  # stationary feats

            # ---- phase 1: warm-up + projections ----
            with (
                tc.tile_pool(name="pwu", bufs=1, space="PSUM") as pwu,
                tc.tile_pool(name="pproj", bufs=1, space="PSUM") as pproj,
                tc.tile_pool(name="pvv", bufs=2, space="PSUM") as pvv,
            ):
                # dummy matmuls bridge the input-DMA wait so the PE HAM
                # clock-gate is released (2.4 GHz) before real work
                wups = pwu.tile([128, 512], F32, tag="wup")
                for i in range(10):
                    nc.tensor.matmul(
                        wups[:], identity[:], warm512[:],
                        start=True, stop=True)

                psq = pproj.tile([128, 256], F32, tag="projq")
                nc.tensor.matmul(psq[:], wq_sb, xqt_sb[:])
                nc.scalar.activation(
                    QTb[:], psq[:], AF.Sigmoid, bias=bq_sb, scale=1.0)
                psk = pproj.tile([128, S], F32, tag="proj")
                nc.tensor.matmul(psk[:, 0:512], wk_sb, xbt_sb[:, 0:512])
                nc.tensor.matmul(psk[:, 512:1024], wk_sb, xbt_sb[:, 512:1024])
                nc.scalar.activation(
                    KTb[:], psk[:], AF.Sigmoid, bias=bk_sb, scale=1.0)
                # preload the exp table set right after the sigmoids
                nc.scalar.activation(warm[:], QTb[:, 0:1], AF.Exp)

                for half in range(2):
                    psv = pvv.tile([128, 4, 128], F32, tag="vv")
                    for j4 in range(4):
                        j = half * 4 + j4
                        nc.tensor.matmul(
                            psv[:, j4, :], xbt_sb[:, ts(j, 128)], wv_sb)
                    nc.scalar.copy(Vn[:, ts(half, 4), :], psv[:])

            # ---- phase 2+3+4: chunked score -> mask+exp -> AV pipeline ----
            with (
                tc.tile_pool(name="utmp", bufs=4) as utp,
                tc.tile_pool(name="psc", bufs=1, space="PSUM") as psc,
                tc.tile_pool(name="ezt", bufs=1) as ezt,
                tc.tile_pool(name="sml", bufs=1) as sml,
                tc.tile_pool(name="pw", bufs=2, space="PSUM") as pw,
                tc.tile_pool(name="po", bufs=1, space="PSUM") as po,
                tc.tile_pool(name="wts", bufs=3) as wtsp,
                tc.tile_pool(name="ob", bufs=2) as ob,
            ):
                QN = utp.tile([128, 256], FP16, tag="qn")
                nc.vector.tensor_scalar(QN[:], QTb[:], -1.0, None, ALU.mult)
                for p in range(P):
                    t_p = p * H
                    # moving: m_p = min(K, t_p) - t_p   [d, k]
                    nc.vector.tensor_scalar(
                        mp[:, p, :], KTb[:], t_p, t_p, ALU.min, ALU.subtract)
                    # stationary: vt_p = min(|Q-t_p|, h) - h
                    #   = max(min(Q-t_p-h, 0), min(-Q+t_p-h, 0))
                    a = utp.tile([128, 256], FP16, tag="ua")
                    b = utp.tile([128, 256], FP16, tag="ub")
                    nc.vector.tensor_scalar(
                        a[:], QTb[:], t_p + H, 0.0, ALU.subtract, ALU.min)
                    nc.vector.tensor_scalar(
                        b[:], QN[:], t_p - H, 0.0, ALU.add, ALU.min)
                    nc.vector.tensor_max(vt[:, p, :], a[:], b[:])

                scA = psc.tile([128, WA], F32, tag="scA")
                scB = psc.tile([128, WB], F32, tag="scB")
                EA = ezt.tile([128, WA], FP16)
                EB = ezt.tile([128, WB], FP16)
                NCA, NCB = WA // 512, WB // 512
                rsA0 = sml.tile([128, 1], F32)
                rsA1 = sml.tile([128, 1], F32)
                rsB0 = sml.tile([128, 1], F32)
                rsB1 = sml.tile([128, 1], F32)
                rs = {("A", 0): rsA0, ("A", 1): rsA1,
                      ("B", 0): rsB0, ("B", 1): rsB1}
                denA = sml.tile([128, 1], F32)
                denB = sml.tile([128, 1], F32)
                rcpA = sml.tile([128, 1], F32)
                rcpB = sml.tile([128, 1], F32)
                oA = po.tile([128, 128], F32, tag="oA")
                oB = po.tile([128, 128], F32, tag="oB")

                chunks = [("A", ca) for ca in range(NCA)]
                chunks += [("B", cb) for cb in range(NCB)]

                def emit_score(tile_id, ci):
                    sc = scA if tile_id == "A" else scB
                    qlo = 0 if tile_id == "A" else 128
                    for p in range(P):
                        nc.tensor.matmul(
                            sc[:, ts(ci, 512)],
                            vt[:, p, qlo:qlo + 128],
                            mp[:, p, ts(ci, 512)],
                            start=(p == 0), stop=(p == P - 1))

                def emit_tail(tile_id, ci, close):
                    sc, E = (scA, EA) if tile_id == "A" else (scB, EB)
                    m1 = m1a_sb if tile_id == "A" else m1b_sb
                    Z = utp.tile([128, 512], FP16, tag="z")
                    nc.vector.scalar_tensor_tensor(
                        out=Z[:], in0=sc[:, ts(ci, 512)], scalar=NEG_DT,
                        in1=m1[:, ts(ci, 512)], op0=ALU.add, op1=ALU.mult)
                    nc.scalar.activation(
                        E[:, ts(ci, 512)], Z[:], AF.Exp, scale=EXP_SCALE,
                        accum_out=rs[(tile_id, ci)][:])
                    o = oA if tile_id == "A" else oB
                    for j4 in range(4):
                        j = ci * 4 + j4
                        pwt = pw.tile([128, 128], FP16, tag="wt")
                        nc.tensor.transpose(
                            pwt[:], E[:, ts(j, 128)], identity[:])
                        wtile = wtsp.tile([128, 128], FP16, tag="wts")
                        if j % 2 == 0:
                            nc.scalar.copy(wtile[:], pwt[:])
                        else:
                            nc.vector.tensor_copy(wtile[:], pwt[:])
                        nc.tensor.matmul(
                            o[:], wtile[:], Vn[:, j, :],
                            start=(j == 0), stop=(close and j4 == 3))

                def emit_out(tile_id, o, rcp, rows):
                    ores = ob.tile([128, 128], F32, tag="ores")
                    nc.vector.scalar_tensor_tensor(
                        out=ores[:], in0=o[:], scalar=rcp[:],
                        in1=bvb_sb, op0=ALU.mult, op1=ALU.add)
                    nc.sync.dma_start(out=out_d[rows, :], in_=ores[:])

                def emit_a_epilogue():
                    if masked:
                        # masked tail keys 512..1023: weight-1
                        for j in range(4, 8):
                            nc.tensor.matmul(
                                oA[:], ones128[:], Vn[:, j, :],
                                start=False, stop=(j == 7))
                        nc.vector.tensor_scalar(
                            denA[:], rs[("A", 0)][:], float(S - WA),
                            None, ALU.add)
                    else:
                        nc.vector.tensor_add(
                            denA[:], rs[("A", 0)][:], rs[("A", 1)][:])
                    nc.vector.reciprocal(rcpA[:], denA[:])
                    emit_out("A", oA, rcpA, slice(0, 128))

                for (tile_id, ci) in chunks:
                    last = ci == (NCA if tile_id == "A" else NCB) - 1
                    emit_score(tile_id, ci)
                    emit_tail(tile_id, ci, close=(last and not
                                                  (tile_id == "A" and masked)))
                    # A's epilogue is emitted between B0 and B1 so it
                    # overlaps B1's score matmuls instead of delaying
                    # B1's Z/exp/AV tail in the engine queues
                    if tile_id == "B" and ci == 0:
                        emit_a_epilogue()
                    if tile_id == "B" and last:
                        nc.vector.tensor_add(
                            denB[:], rs[("B", 0)][:], rs[("B", 1)][:])
                        nc.vector.reciprocal(rcpB[:], denB[:])
                        emit_out("B", oB, rcpB, slice(128, 256))

    nc.finalize()
    return nc


_PROG_CACHE: dict[bool, bass.Bass] = {}


def _get_program(masked: bool) -> bass.Bass:
    if masked not in _PROG_CACHE:
        _PROG_CACHE[masked] = _build_program(masked)
    return _PROG_CACHE[masked]


def build_in_maps(x, Wq, bq, Wk, bk, Wv, bv, masked):
    wkt = np.ascontiguousarray(Wk.T.astype(np.float16))
    wqt = np.ascontiguousarray(Wq.T.astype(np.float16))
    wvt = np.ascontiguousarray(Wv.T.astype(np.float16))
    bcat = np.ascontiguousarray(
        np.concatenate(
            [bk.reshape(D, 1), bq.reshape(D, 1),
             np.tile(bv.reshape(1, D), (D, 1))], axis=1).astype(np.float32))
    WA = 512 if masked else 1024
    WB = 1024
    kidx = np.arange(S)
    in_maps = []
    for c in range(NCORES):
        b, l = divmod(c, 4)
        xb16 = x[b].astype(np.float16)
        xbt = np.ascontiguousarray(xb16.T)
        rows = np.concatenate(
            [128 * l + np.arange(128), 128 * (4 + l) + np.arange(128)])
        xqt = np.ascontiguousarray(xb16[rows].T)
        if masked:
            qa = (128 * l + np.arange(128))[:, None]
            qb = (128 * (4 + l) + np.arange(128))[:, None]
            m1a = (kidx[None, :WA] <= qa).astype(np.float16)
            m1b = (kidx[None, :WB] <= qb).astype(np.float16)
        else:
            m1a = np.ones((128, WA), np.float16)
            m1b = np.ones((128, WB), np.float16)
        in_maps.append({
            "xbt": xbt, "xqt": xqt, "wk": wkt, "wq": wqt, "wv": wvt,
            "bcat": bcat,
            "m1a": np.ascontiguousarray(m1a),
            "m1b": np.ascontiguousarray(m1b),
        })
    return in_maps


def assemble_out(results, masked):
    out = np.empty((B, S, D), dtype=np.float32)
    for c in range(NCORES):
        b, l = divmod(c, 4)
        res = results[c]["out"]
        out[b, 128 * l:128 * (l + 1)] = res[0:128]
        out[b, 128 * (4 + l):128 * (5 + l)] = res[128:256]
    return out


def kernel(x, Wq, bq, Wk, bk, Wv, bv, apply_causal_mask):
    x = np.ascontiguousarray(np.asarray(x, dtype=np.float32))
    Wq = np.asarray(Wq, dtype=np.float32)
    Wk = np.asarray(Wk, dtype=np.float32)
    Wv = np.asarray(Wv, dtype=np.float32)
    bq = np.asarray(bq, dtype=np.float32)
    bk = np.asarray(bk, dtype=np.float32)
    bv = np.asarray(bv, dtype=np.float32)
    masked = bool(int(np.asarray(apply_causal_mask)))

    nc = _get_program(masked)
    in_maps = build_in_maps(x, Wq, bq, Wk, bk, Wv, bv, masked)
    res = run_bass_kernel_spmd(nc, in_maps, list(range(NCORES))).results
    return assemble_out(res, masked)


# revision 20
# speedup vs baseline: 2.7826x; 1.0900x over previous
"""Trainium2 Bass kernel for DifferentiableToposAttention.

Math:
  Q = sigmoid(x @ Wq.T + bq); K = sigmoid(x @ Wk.T + bk); V = x @ Wv.T + bv
  truth[q,k] = 1 - (1/D) sum_d relu(Q[q,d]-K[k,d]);  logit = 10*truth
  masked (k>q) positions get logit 0 exactly (softmax weight exp(0)=1).

Algorithmic core: piecewise-linear feature factorization.  With knots
t_p = p/T (p=0..T, h=1/T) and hat functions phi_p (interpolation in the
K variable is exact between knots; only the cell containing the kink of
relu carries O(h^2) error):

  relu(a-b) ~= sum_p phi_p(a) * relu(t_p - b)
  phi_p(a)  = -T * vt_p(a),  vt_p(a) = min(|a - t_p|, h) - h
  relu(t_p-b) = -(min(b, t_p) - t_p) = -m_p(b)

  sum_d relu(Q-K) ~= T * sum_{d,p} vt_p(Q[q,d]) * m_p(K[k,d]) =: T * SC

so the whole pairwise nonlinearity becomes one dense matmul with
contraction dim D*(T+1), run at 128x128 MACs/cycle on the PE instead of
the 128/cycle of a partition reduce.  logit = 10 - (10T/D)*SC.

Masking uses Z = (SC - D/T) * M1 (M1 host-supplied 0/1), so that
E = exp(-10T/D * Z) gives exp(logit) unmasked and exp(0)=1 masked, with
no bias corrections.

Sharding: 8 cores; core c = (b, l) = (c//4, c%4) handles batch b, query
tiles l (keys 0..511 computed) and 4+l (keys 0..1023).  Shapes are
identical across cores (SPMD); causality is entirely in the M1 mask
data.  Keys >= 512 for tile A are all masked: weight-1 contributions
come from an all-ones stationary over V blocks 4..7 plus a +512
denominator constant.

Pipelining: score PSUM is built in 512-wide chunks in order A, B0, B1;
each chunk's Z -> exp -> EtT transposes -> AV matmuls overlap the next
chunk's score matmuls.
"""

import sys

for _p in ("/opt/trn_rl_repo",):
    if _p not in sys.path:
        sys.path.insert(0, _p)

import numpy as np

import concourse.bass as bass
import concourse.mybir as mybir
import concourse.tile as tile
from concourse import bacc
from concourse.bass import ts
from concourse.masks import make_identity
from concourse.bass_utils import run_bass_kernel_spmd

F32 = mybir.dt.float32
FP16 = mybir.dt.float16
AF = mybir.ActivationFunctionType
ALU = mybir.AluOpType

B, S, D = 2, 1024, 128
NCORES = 8
T = 6                    # knot count (h = 1/T); P = T+1 features per d
P = T + 1
H = 1.0 / T
POS_DT = float(D) / T    # Z = (SC + POS_DT) * M1  (SC <= 0)
EXP_SCALE = 10.0 * T / D
XKNOTS = (P - 2, P - 1)  # moving feature on ACT (relu) for these knots


def _build_program(masked: bool) -> bass.Bass:
    WA = 512 if masked else 1024   # computed key width, query tile A (tile l)
    WB = 1024                      # query tile B (tile 4+l)
    nc = bacc.Bacc()

    xbt_d = nc.declare_dram_parameter("xbt", [D, S], FP16, isOutput=False)
    xqt_d = nc.declare_dram_parameter("xqt", [D, 256], FP16, isOutput=False)
    wk_d = nc.declare_dram_parameter("wk", [D, D], FP16, isOutput=False)
    wq_d = nc.declare_dram_parameter("wq", [D, D], FP16, isOutput=False)
    wv_d = nc.declare_dram_parameter("wv", [D, D], FP16, isOutput=False)
    # bcat = [bk | bq | bvb(128 cols)]
    bcat_d = nc.declare_dram_parameter("bcat", [D, 2 + D], F32, isOutput=False)
    m1a_d = nc.declare_dram_parameter("m1a", [D, WA], FP16, isOutput=False)
    m1b_d = nc.declare_dram_parameter("m1b", [D, WB], FP16, isOutput=False)
    out_d = nc.declare_dram_parameter("out", [256, D], F32, isOutput=True)

    with tile.TileContext(nc) as tc:
        with tc.tile_pool(name="singles", bufs=1) as singles:
            wk_sb = singles.tile([128, 128], FP16)
            wq_sb = singles.tile([128, 128], FP16)
            wv_sb = singles.tile([128, 128], FP16)
            bcat_sb = singles.tile([128, 2 + 128], F32)
            xbt_sb = singles.tile([128, S], FP16)
            xqt_sb = singles.tile([128, 256], FP16)
            m1a_sb = singles.tile([128, WA], FP16)
            m1b_sb = singles.tile([128, WB], FP16)

            identity = singles.tile([128, 128], FP16)
            make_identity(nc, identity[:])
            warm512 = singles.tile([128, 512], FP16)
            nc.vector.memset(warm512[:], 1.0)
            ones128 = singles.tile([128, 128], FP16)
            nc.vector.memset(ones128[:], 1.0)

            # input DMAs spread across engine queues so they complete in
            # parallel; x/weights (needed first) lead.  The scalar queue
            # carries none so its ACT table load runs immediately.
            nc.sync.dma_start(out=xbt_sb[:], in_=xbt_d[:, :])
            nc.sync.dma_start(out=xqt_sb[:], in_=xqt_d[:, :])
            nc.gpsimd.dma_start(out=wq_sb[:], in_=wq_d[:, :])
            nc.gpsimd.dma_start(out=wk_sb[:], in_=wk_d[:, :])
            nc.gpsimd.dma_start(out=wv_sb[:], in_=wv_d[:, :])
            nc.gpsimd.dma_start(out=bcat_sb[:], in_=bcat_d[:, :])
            nc.sync.dma_start(out=m1a_sb[:], in_=m1a_d[:, :])
            nc.sync.dma_start(out=m1b_sb[:], in_=m1b_d[:, :])

            bk_sb = bcat_sb[:, 0:1]
            bq_sb = bcat_sb[:, 1:2]
            bvb_sb = bcat_sb[:, 2:130]

            warm = singles.tile([128, 1], F32)
            # pull the sigmoid ACT table load to t~0 (no data deps)
            nc.scalar.activation(warm[:], ones128[:, 0:1], AF.Sigmoid)
            # per-partition bias constants t_p for the ACT relu knots
            tkn = singles.tile([128, len(XKNOTS)], F32)
            for i, p in enumerate(XKNOTS):
                nc.vector.memset(tkn[:, i:i + 1], p * H)

            KTb = singles.tile([128, S], FP16)     # sigmoid K^T  [d, k]
            QTb = singles.tile([128, 256], FP16)   # sigmoid Q^T  [d, q]
            Vn = singles.tile([128, 8, 128], FP16)  # V (no bv)  [k, blk, e]
            mp = singles.tile([128, P, S], FP16)    # moving feats min(K,t)-t
            vt = singles.tile([128, P, 256], FP16)  # stationary feats

            # ---- phase 1: warm-up + K/Q projections ----
            with (
                tc.tile_pool(name="pwu", bufs=1, space="PSUM") as pwu,
                tc.tile_pool(name="pproj", bufs=1, space="PSUM") as pproj,
            ):
                # dummy matmuls bridge the input-DMA wait so the PE HAM
                # clock-gate is released (2.4 GHz) before real work
                wups = pwu.tile([128, 512], F32, tag="wup")
                for i in range(8):
                    nc.tensor.matmul(
                        wups[:], identity[:], warm512[:],
                        start=True, stop=True)

                psq = pproj.tile([128, 256], F32, tag="projq")
                nc.tensor.matmul(psq[:], wq_sb, xqt_sb[:])
                nc.scalar.activation(
                    QTb[:], psq[:], AF.Sigmoid, bias=bq_sb, scale=1.0)
                psk = pproj.tile([128, S], F32, tag="proj")
                for hh in range(2):
                    nc.tensor.matmul(
                        psk[:, ts(hh, 512)], wk_sb, xbt_sb[:, ts(hh, 512)])
                    nc.scalar.activation(
                        KTb[:, ts(hh, 512)], psk[:, ts(hh, 512)],
                        AF.Sigmoid, bias=bk_sb, scale=1.0)

            # ---- phase 2+3+4: features, chunked score -> exp -> AV ----
            with (
                tc.tile_pool(name="pvv", bufs=1, space="PSUM") as pvv,
                tc.tile_pool(name="utmp", bufs=4) as utp,
                tc.tile_pool(name="psc", bufs=1, space="PSUM") as psc,
                tc.tile_pool(name="ezt", bufs=1) as ezt,
                tc.tile_pool(name="sml", bufs=1) as sml,
                tc.tile_pool(name="pw", bufs=2, space="PSUM") as pw,
                tc.tile_pool(name="po", bufs=1, space="PSUM") as po,
                tc.tile_pool(name="wts", bufs=3) as wtsp,
                tc.tile_pool(name="ob", bufs=2) as ob,
            ):
                QN = utp.tile([128, 256], FP16, tag="qn")
                nc.vector.tensor_scalar(QN[:], QTb[:], -1.0, None, ALU.mult)
                for p in range(P):
                    t_p = p * H
                    if p in XKNOTS:
                        # moving on ACT: g_p = relu(t_p - K) >= 0
                        tp_ap = tkn[:, XKNOTS.index(p):XKNOTS.index(p) + 1]
                        for hh in range(2):
                            nc.scalar.activation(
                                mp[:, p, ts(hh, 512)], KTb[:, ts(hh, 512)],
                                AF.Relu, bias=tp_ap, scale=-1.0)
                        # stationary vtNEG_p = max(min(Q-t_p-h,0),
                        #                          min(-Q+t_p-h,0)) <= 0
                        a = utp.tile([128, 256], FP16, tag="ua")
                        b = utp.tile([128, 256], FP16, tag="ub")
                        nc.vector.tensor_scalar(
                            a[:], QTb[:], t_p + H, 0.0, ALU.subtract, ALU.min)
                        nc.vector.tensor_scalar(
                            b[:], QN[:], t_p - H, 0.0, ALU.add, ALU.min)
                        nc.vector.tensor_max(vt[:, p, :], a[:], b[:])
                    else:
                        # moving on DVE: m_p = min(K, t_p) - t_p <= 0
                        for hh in range(2):
                            nc.vector.tensor_scalar(
                                mp[:, p, ts(hh, 512)], KTb[:, ts(hh, 512)],
                                t_p, t_p, ALU.min, ALU.subtract)
                        # stationary vtPOS_p = min(relu(t_p+h-Q),
                        #                          relu(Q-t_p+h)) >= 0
                        a = utp.tile([128, 256], FP16, tag="ua")
                        b = utp.tile([128, 256], FP16, tag="ub")
                        nc.vector.tensor_scalar(
                            a[:], QN[:], t_p + H, 0.0, ALU.add, ALU.max)
                        nc.vector.tensor_scalar(
                            b[:], QTb[:], t_p - H, 0.0, ALU.subtract, ALU.max)
                        nc.vector.tensor_tensor(
                            vt[:, p, :], a[:], b[:], ALU.min)

                # V projection + copies; exp table preload
                for half in range(2):
                    psv = pvv.tile([128, 4, 128], F32, tag="vv")
                    for j4 in range(4):
                        j = half * 4 + j4
                        nc.tensor.matmul(
                            psv[:, j4, :], xbt_sb[:, ts(j, 128)], wv_sb)
                    nc.scalar.copy(Vn[:, ts(half, 4), :], psv[:])
                nc.scalar.activation(warm[:], QTb[:, 0:1], AF.Exp)

                scA = psc.tile([128, WA], F32, tag="scA")
                scB = psc.tile([128, WB], F32, tag="scB")
                EA = ezt.tile([128, WA], FP16)
                EB = ezt.tile([128, WB], FP16)
                NCA, NCB = WA // 512, WB // 512
                rsA0 = sml.tile([128, 1], F32)
                rsA1 = sml.tile([128, 1], F32)
                rsB0 = sml.tile([128, 1], F32)
                rsB1 = sml.tile([128, 1], F32)
                rs2 = sml.tile([128, 1], F32)
                rs = {("A", 0): rsA0, ("A", 1): rsA1,
                      ("B", 0): rsB0, ("B", 1): rsB1}
                denA = sml.tile([128, 1], F32)
                denB = sml.tile([128, 1], F32)
                rcpA = sml.tile([128, 1], F32)
                rcpB = sml.tile([128, 1], F32)
                oA = po.tile([128, 128], F32, tag="oA")
                oB = po.tile([128, 128], F32, tag="oB")

                chunks = [("A", ca) for ca in range(NCA)]
                chunks += [("B", cb) for cb in range(NCB)]

                def emit_score(tile_id, ci):
                    sc = scA if tile_id == "A" else scB
                    qlo = 0 if tile_id == "A" else 128
                    for p in range(P):
                        nc.tensor.matmul(
                            sc[:, ts(ci, 512)],
                            vt[:, p, qlo:qlo + 128],
                            mp[:, p, ts(ci, 512)],
                            start=(p == 0), stop=(p == P - 1))

                def emit_tail(tile_id, ci, close):
                    sc, E = (scA, EA) if tile_id == "A" else (scB, EB)
                    m1 = m1a_sb if tile_id == "A" else m1b_sb
                    Z = utp.tile([128, 512], FP16, tag="z")
                    nc.vector.scalar_tensor_tensor(
                        out=Z[:], in0=sc[:, ts(ci, 512)], scalar=POS_DT,
                        in1=m1[:, ts(ci, 512)], op0=ALU.add, op1=ALU.mult)
                    o = oA if tile_id == "A" else oB
                    # exp in halves on the closing chunk so the first two
                    # transposes start ~0.4us earlier
                    nhalf = 2 if close else 1
                    rsc = rs[(tile_id, ci)]
                    for eh in range(nhalf):
                        w2 = 512 // nhalf
                        racc = rsc if eh == 0 else rs2
                        nc.scalar.activation(
                            E[:, ci * 512 + eh * w2:ci * 512 + (eh + 1) * w2],
                            Z[:, eh * w2:(eh + 1) * w2], AF.Exp,
                            scale=EXP_SCALE, accum_out=racc[:])
                        for j4 in range(eh * 4 // nhalf,
                                        (eh + 1) * 4 // nhalf):
                            j = ci * 4 + j4
                            pwt = pw.tile([128, 128], FP16, tag="wt")
                            nc.tensor.transpose(
                                pwt[:], E[:, ts(j, 128)], identity[:])
                            wtile = wtsp.tile([128, 128], FP16, tag="wts")
                            if j % 2 == 0:
                                nc.scalar.copy(wtile[:], pwt[:])
                            else:
                                nc.vector.tensor_copy(wtile[:], pwt[:])
                            nc.tensor.matmul(
                                o[:], wtile[:], Vn[:, j, :],
                                start=(j == 0), stop=(close and j4 == 3))
                    if nhalf == 2:
                        nc.vector.tensor_add(rsc[:], rsc[:], rs2[:])

                def emit_out(tile_id, o, rcp, rows):
                    ores = ob.tile([128, 128], F32, tag="ores")
                    nc.vector.scalar_tensor_tensor(
                        out=ores[:], in0=o[:], scalar=rcp[:],
                        in1=bvb_sb, op0=ALU.mult, op1=ALU.add)
                    nc.sync.dma_start(out=out_d[rows, :], in_=ores[:])

                def emit_a_epilogue():
                    if masked:
                        # masked tail keys 512..1023: weight-1
                        for j in range(4, 8):
                            nc.tensor.matmul(
                                oA[:], ones128[:], Vn[:, j, :],
                                start=False, stop=(j == 7))
                        nc.vector.tensor_scalar(
                            denA[:], rs[("A", 0)][:], float(S - WA),
                            None, ALU.add)
                    else:
                        nc.vector.tensor_add(
                            denA[:], rs[("A", 0)][:], rs[("A", 1)][:])
                    nc.vector.reciprocal(rcpA[:], denA[:])
                    emit_out("A", oA, rcpA, slice(0, 128))

                for (tile_id, ci) in chunks:
                    last = ci == (NCA if tile_id == "A" else NCB) - 1
                    emit_score(tile_id, ci)
                    emit_tail(tile_id, ci, close=(last and not
                                                  (tile_id == "A" and masked)))
                    # A's epilogue is emitted between B0 and B1 so it
                    # overlaps B1's score matmuls instead of delaying
                    # B1's Z/exp/AV tail in the engine queues
                    if tile_id == "B" and ci == 0:
                        emit_a_epilogue()
                    if tile_id == "B" and last:
                        nc.vector.tensor_add(
                            denB[:], rs[("B", 0)][:], rs[("B", 1)][:])
                        nc.vector.reciprocal(rcpB[:], denB[:])
                        emit_out("B", oB, rcpB, slice(128, 256))

    nc.finalize()
    return nc


_PROG_CACHE: dict[bool, bass.Bass] = {}


def _get_program(masked: bool) -> bass.Bass:
    if masked not in _PROG_CACHE:
        _PROG_CACHE[masked] = _build_program(masked)
    return _PROG_CACHE[masked]


def build_in_maps(x, Wq, bq, Wk, bk, Wv, bv, masked):
    wkt = np.ascontiguousarray(Wk.T.astype(np.float16))
    wqt = np.ascontiguousarray(Wq.T.astype(np.float16))
    wvt = np.ascontiguousarray(Wv.T.astype(np.float16))
    bcat = np.ascontiguousarray(
        np.concatenate(
            [bk.reshape(D, 1), bq.reshape(D, 1),
             np.tile(bv.reshape(1, D), (D, 1))], axis=1).astype(np.float32))
    WA = 512 if masked else 1024
    WB = 1024
    kidx = np.arange(S)
    in_maps = []
    for c in range(NCORES):
        b, l = divmod(c, 4)
        xb16 = x[b].astype(np.float16)
        xbt = np.ascontiguousarray(xb16.T)
        rows = np.concatenate(
            [128 * l + np.arange(128), 128 * (4 + l) + np.arange(128)])
        xqt = np.ascontiguousarray(xb16[rows].T)
        if masked:
            qa = (128 * l + np.arange(128))[:, None]
            qb = (128 * (4 + l) + np.arange(128))[:, None]
            m1a = (kidx[None, :WA] <= qa).astype(np.float16)
            m1b = (kidx[None, :WB] <= qb).astype(np.float16)
        else:
            m1a = np.ones((128, WA), np.float16)
            m1b = np.ones((128, WB), np.float16)
        in_maps.append({
            "xbt": xbt, "xqt": xqt, "wk": wkt, "wq": wqt, "wv": wvt,
            "bcat": bcat,
            "m1a": np.ascontiguousarray(m1a),
            "m1b": np.ascontiguousarray(m1b),
        })
    return in_maps


def assemble_out(results, masked):
    out = np.empty((B, S, D), dtype=np.float32)
    for c in range(NCORES):
        b, l = divmod(c, 4)
        res = results[c]["out"]
        out[b, 128 * l:128 * (l + 1)] = res[0:128]
        out[b, 128 * (4 + l):128 * (5 + l)] = res[128:256]
    return out


def kernel(x, Wq, bq, Wk, bk, Wv, bv, apply_causal_mask):
    x = np.ascontiguousarray(np.asarray(x, dtype=np.float32))
    Wq = np.asarray(Wq, dtype=np.float32)
    Wk = np.asarray(Wk, dtype=np.float32)
    Wv = np.asarray(Wv, dtype=np.float32)
    bq = np.asarray(bq, dtype=np.float32)
    bk = np.asarray(bk, dtype=np.float32)
    bv = np.asarray(bv, dtype=np.float32)
    masked = bool(int(np.asarray(apply_causal_mask)))

    nc = _get_program(masked)
    in_maps = build_in_maps(x, Wq, bq, Wk, bk, Wv, bv, masked)
    res = run_bass_kernel_spmd(nc, in_maps, list(range(NCORES))).results
    return assemble_out(res, masked)


# revision 24
# speedup vs baseline: 2.8962x; 1.0408x over previous
"""Trainium2 Bass kernel for DifferentiableToposAttention.

Math:
  Q = sigmoid(x @ Wq.T + bq); K = sigmoid(x @ Wk.T + bk); V = x @ Wv.T + bv
  truth[q,k] = 1 - (1/D) sum_d relu(Q[q,d]-K[k,d]);  logit = 10*truth
  masked (k>q) positions get logit 0 exactly (softmax weight exp(0)=1).

Algorithmic core: piecewise-linear feature factorization.  With knots
t_p = p/T (p=0..T, h=1/T) and hat functions phi_p (interpolation in the
K variable is exact between knots; only the cell containing the kink of
relu carries O(h^2) error):

  relu(a-b) ~= sum_p phi_p(a) * relu(t_p - b)
  phi_p(a)  = -T * vt_p(a),  vt_p(a) = min(|a - t_p|, h) - h
  relu(t_p-b) = -(min(b, t_p) - t_p) = -m_p(b)

  sum_d relu(Q-K) ~= T * sum_{d,p} vt_p(Q[q,d]) * m_p(K[k,d]) =: T * SC

so the whole pairwise nonlinearity becomes one dense matmul with
contraction dim D*(T+1), run at 128x128 MACs/cycle on the PE instead of
the 128/cycle of a partition reduce.  logit = 10 - (10T/D)*SC.

Masking uses Z = (SC - D/T) * M1 (M1 host-supplied 0/1), so that
E = exp(-10T/D * Z) gives exp(logit) unmasked and exp(0)=1 masked, with
no bias corrections.

Sharding: 8 cores; core c = (b, l) = (c//4, c%4) handles batch b, query
tiles l (keys 0..511 computed) and 4+l (keys 0..1023).  Shapes are
identical across cores (SPMD); causality is entirely in the M1 mask
data.  Keys >= 512 for tile A are all masked: weight-1 contributions
come from an all-ones stationary over V blocks 4..7 plus a +512
denominator constant.

Pipelining: score PSUM is built in 512-wide chunks in order A, B0, B1;
each chunk's Z -> exp -> EtT transposes -> AV matmuls overlap the next
chunk's score matmuls.
"""

import sys

for _p in ("/opt/trn_rl_repo",):
    if _p not in sys.path:
        sys.path.insert(0, _p)

import numpy as np

import concourse.bass as bass
import concourse.mybir as mybir
import concourse.tile as tile
from concourse import bacc
from concourse.bass import ts
from concourse.masks import make_identity
from concourse.bass_utils import run_bass_kernel_spmd

F32 = mybir.dt.float32
FP16 = mybir.dt.float16
AF = mybir.ActivationFunctionType
ALU = mybir.AluOpType

B, S, D = 2, 1024, 128
NCORES = 8
T = 6                    # knot count (h = 1/T); P = T+1 features per d
P = T + 1
H = 1.0 / T
POS_DT = float(D) / T    # Z = (SC + POS_DT) * M1  (SC <= 0)
EXP_SCALE = 10.0 * T / D
XKNOTS = (P - 2, P - 1)  # moving feature on ACT (relu) for these knots


def _build_program(masked: bool) -> bass.Bass:
    WA = 512 if masked else 1024   # computed key width, query tile A (tile l)
    WB = 1024                      # query tile B (tile 4+l)
    nc = bacc.Bacc()

    xbt_d = nc.declare_dram_parameter("xbt", [D, S], FP16, isOutput=False)
    xqt_d = nc.declare_dram_parameter("xqt", [D, 256], FP16, isOutput=False)
    wk_d = nc.declare_dram_parameter("wk", [D, D], FP16, isOutput=False)
    wq_d = nc.declare_dram_parameter("wq", [D, D], FP16, isOutput=False)
    wv_d = nc.declare_dram_parameter("wv", [D, D], FP16, isOutput=False)
    # bcat = [bk | bq | bvb(128 cols)]
    bcat_d = nc.declare_dram_parameter("bcat", [D, 2 + D], F32, isOutput=False)
    m1a_d = nc.declare_dram_parameter("m1a", [D, WA], FP16, isOutput=False)
    m1b_d = nc.declare_dram_parameter("m1b", [D, WB], FP16, isOutput=False)
    out_d = nc.declare_dram_parameter("out", [256, D], F32, isOutput=True)

    with tile.TileContext(nc) as tc:
        with tc.tile_pool(name="singles", bufs=1) as singles:
            wk_sb = singles.tile([128, 128], FP16)
            wq_sb = singles.tile([128, 128], FP16)
            wv_sb = singles.tile([128, 128], FP16)
            bcat_sb = singles.tile([128, 2 + 128], F32)
            xbt_sb = singles.tile([128, S], FP16)
            xqt_sb = singles.tile([128, 256], FP16)
            m1a_sb = singles.tile([128, WA], FP16)
            m1b_sb = singles.tile([128, WB], FP16)

            identity = singles.tile([128, 128], FP16)
            make_identity(nc, identity[:])
            warm512 = singles.tile([128, 512], FP16)
            nc.vector.memset(warm512[:], 1.0)
            ones128 = singles.tile([128, 128], FP16)
            nc.vector.memset(ones128[:], 1.0)

            # input DMAs spread across engine queues so they complete in
            # parallel; Q-path inputs (first consumers) lead their queues
            nc.sync.dma_start(out=xqt_sb[:], in_=xqt_d[:, :])
            nc.sync.dma_start(out=xbt_sb[:], in_=xbt_d[:, :])
            nc.scalar.dma_start(out=wq_sb[:], in_=wq_d[:, :])
            nc.scalar.dma_start(out=wk_sb[:], in_=wk_d[:, :])
            nc.gpsimd.dma_start(out=bcat_sb[:], in_=bcat_d[:, :])
            nc.gpsimd.dma_start(out=wv_sb[:], in_=wv_d[:, :])
            nc.sync.dma_start(out=m1a_sb[:], in_=m1a_d[:, :])
            nc.sync.dma_start(out=m1b_sb[:], in_=m1b_d[:, :])

            bk_sb = bcat_sb[:, 0:1]
            bq_sb = bcat_sb[:, 1:2]
            bvb_sb = bcat_sb[:, 2:130]

            warm = singles.tile([128, 1], F32)
            # pull the sigmoid ACT table load to t~0 (no data deps)
            nc.scalar.activation(warm[:], ones128[:, 0:1], AF.Sigmoid)
            # per-partition bias constants t_p for the ACT relu knots
            tkn = singles.tile([128, len(XKNOTS)], F32)
            for i, p in enumerate(XKNOTS):
                nc.vector.memset(tkn[:, i:i + 1], p * H)

            KTb = singles.tile([128, S], FP16)     # sigmoid K^T  [d, k]
            QTb = singles.tile([128, 256], FP16)   # sigmoid Q^T  [d, q]
            Vn = singles.tile([128, 8, 128], FP16)  # V (no bv)  [k, blk, e]
            mp = singles.tile([128, P, S], FP16)    # moving feats min(K,t)-t
            vt = singles.tile([128, P, 256], FP16)  # stationary feats

            # ---- phase 1: warm-up + K/Q projections ----
            with (
                tc.tile_pool(name="pwu", bufs=1, space="PSUM") as pwu,
                tc.tile_pool(name="pproj", bufs=1, space="PSUM") as pproj,
            ):
                # dummy matmuls bridge the input-DMA wait so the PE HAM
                # clock-gate is released (2.4 GHz) before real work
                wups = pwu.tile([128, 512], F32, tag="wup")
                for i in range(5):
                    nc.tensor.matmul(
                        wups[:], identity[:], warm512[:],
                        start=True, stop=True)

                psq = pproj.tile([128, 256], F32, tag="projq")
                nc.tensor.matmul(psq[:], wq_sb, xqt_sb[:])
                nc.scalar.activation(
                    QTb[:], psq[:], AF.Sigmoid, bias=bq_sb, scale=1.0)
                psk = pproj.tile([128, S], F32, tag="proj")
                for hh in range(2):
                    nc.tensor.matmul(
                        psk[:, ts(hh, 512)], wk_sb, xbt_sb[:, ts(hh, 512)])
                    nc.scalar.activation(
                        KTb[:, ts(hh, 512)], psk[:, ts(hh, 512)],
                        AF.Sigmoid, bias=bk_sb, scale=1.0)

            # ---- phase 2+3+4: features, chunked score -> exp -> AV ----
            with (
                tc.tile_pool(name="pvv", bufs=1, space="PSUM") as pvv,
                tc.tile_pool(name="utmp", bufs=4) as utp,
                tc.tile_pool(name="psc", bufs=1, space="PSUM") as psc,
                tc.tile_pool(name="ezt", bufs=1) as ezt,
                tc.tile_pool(name="sml", bufs=1) as sml,
                tc.tile_pool(name="pw", bufs=2, space="PSUM") as pw,
                tc.tile_pool(name="po", bufs=1, space="PSUM") as po,
                tc.tile_pool(name="wts", bufs=3) as wtsp,
                tc.tile_pool(name="ob", bufs=2) as ob,
            ):
                QN = utp.tile([128, 256], FP16, tag="qn")
                nc.vector.tensor_scalar(QN[:], QTb[:], -1.0, None, ALU.mult)
                for p in range(P):
                    t_p = p * H
                    if p in XKNOTS:
                        # moving on ACT: g_p = relu(t_p - K) >= 0
                        tp_ap = tkn[:, XKNOTS.index(p):XKNOTS.index(p) + 1]
                        for hh in range(2):
                            nc.scalar.activation(
                                mp[:, p, ts(hh, 512)], KTb[:, ts(hh, 512)],
                                AF.Relu, bias=tp_ap, scale=-1.0)
                        # stationary vtNEG_p = max(min(Q-t_p-h,0),
                        #                          min(-Q+t_p-h,0)) <= 0
                        a = utp.tile([128, 256], FP16, tag="ua")
                        b = utp.tile([128, 256], FP16, tag="ub")
                        nc.vector.tensor_scalar(
                            a[:], QTb[:], t_p + H, 0.0, ALU.subtract, ALU.min)
                        nc.vector.tensor_scalar(
                            b[:], QN[:], t_p - H, 0.0, ALU.add, ALU.min)
                        nc.vector.tensor_max(vt[:, p, :], a[:], b[:])
                    else:
                        # moving on DVE: m_p = min(K, t_p) - t_p <= 0
                        for hh in range(2):
                            nc.vector.tensor_scalar(
                                mp[:, p, ts(hh, 512)], KTb[:, ts(hh, 512)],
                                t_p, t_p, ALU.min, ALU.subtract)
                        # stationary vtPOS_p = min(relu(t_p+h-Q),
                        #                          relu(Q-t_p+h)) >= 0
                        a = utp.tile([128, 256], FP16, tag="ua")
                        b = utp.tile([128, 256], FP16, tag="ub")
                        nc.vector.tensor_scalar(
                            a[:], QN[:], t_p + H, 0.0, ALU.add, ALU.max)
                        nc.vector.tensor_scalar(
                            b[:], QTb[:], t_p - H, 0.0, ALU.subtract, ALU.max)
                        nc.vector.tensor_tensor(
                            vt[:, p, :], a[:], b[:], ALU.min)

                # V projection + copies; exp table preload
                for half in range(2):
                    psv = pvv.tile([128, 4, 128], F32, tag="vv")
                    for j4 in range(4):
                        j = half * 4 + j4
                        nc.tensor.matmul(
                            psv[:, j4, :], xbt_sb[:, ts(j, 128)], wv_sb)
                    nc.scalar.copy(Vn[:, ts(half, 4), :], psv[:])
                nc.scalar.activation(warm[:], QTb[:, 0:1], AF.Exp)

                scA = psc.tile([128, WA], F32, tag="scA")
                scB = psc.tile([128, WB], F32, tag="scB")
                EA = ezt.tile([128, WA], FP16)
                EB = ezt.tile([128, WB], FP16)
                NCA, NCB = WA // 512, WB // 512
                rsA0 = sml.tile([128, 1], F32)
                rsA1 = sml.tile([128, 1], F32)
                rsB0 = sml.tile([128, 1], F32)
                rsB1 = sml.tile([128, 1], F32)
                rs2 = sml.tile([128, 1], F32)
                rs = {("A", 0): rsA0, ("A", 1): rsA1,
                      ("B", 0): rsB0, ("B", 1): rsB1}
                denA = sml.tile([128, 1], F32)
                denB = sml.tile([128, 1], F32)
                rcpA = sml.tile([128, 1], F32)
                rcpB = sml.tile([128, 1], F32)
                oA = po.tile([128, 128], F32, tag="oA")
                oB = po.tile([128, 128], F32, tag="oB")

                chunks = [("A", ca) for ca in range(NCA)]
                chunks += [("B", cb) for cb in range(NCB)]

                def emit_score(tile_id, ci):
                    sc = scA if tile_id == "A" else scB
                    qlo = 0 if tile_id == "A" else 128
                    for p in range(P):
                        nc.tensor.matmul(
                            sc[:, ts(ci, 512)],
                            vt[:, p, qlo:qlo + 128],
                            mp[:, p, ts(ci, 512)],
                            start=(p == 0), stop=(p == P - 1))

                def emit_zexp(tile_id, ci, nhalf):
                    sc, E = (scA, EA) if tile_id == "A" else (scB, EB)
                    m1 = m1a_sb if tile_id == "A" else m1b_sb
                    Z = utp.tile([128, 512], FP16, tag="z")
                    nc.vector.scalar_tensor_tensor(
                        out=Z[:], in0=sc[:, ts(ci, 512)], scalar=POS_DT,
                        in1=m1[:, ts(ci, 512)], op0=ALU.add, op1=ALU.mult)
                    rsc = rs[(tile_id, ci)]
                    for eh in range(nhalf):
                        w2 = 512 // nhalf
                        racc = rsc if eh == 0 else rs2
                        nc.scalar.activation(
                            E[:, ci * 512 + eh * w2:ci * 512 + (eh + 1) * w2],
                            Z[:, eh * w2:(eh + 1) * w2], AF.Exp,
                            scale=EXP_SCALE, accum_out=racc[:])
                    if nhalf == 2:
                        nc.vector.tensor_add(rsc[:], rsc[:], rs2[:])

                def emit_trav(tile_id, ci, close):
                    E = EA if tile_id == "A" else EB
                    o = oA if tile_id == "A" else oB
                    for j4 in range(4):
                        j = ci * 4 + j4
                        pwt = pw.tile([128, 128], FP16, tag="wt")
                        nc.tensor.transpose(
                            pwt[:], E[:, ts(j, 128)], identity[:])
                        wtile = wtsp.tile([128, 128], FP16, tag="wts")
                        if j % 2 == 0:
                            nc.scalar.copy(wtile[:], pwt[:])
                        else:
                            nc.vector.tensor_copy(wtile[:], pwt[:])
                        nc.tensor.matmul(
                            o[:], wtile[:], Vn[:, j, :],
                            start=(j == 0), stop=(close and j4 == 3))

                def emit_out(tile_id, o, rcp, rows):
                    ores = ob.tile([128, 128], F32, tag="ores")
                    nc.vector.scalar_tensor_tensor(
                        out=ores[:], in0=o[:], scalar=rcp[:],
                        in1=bvb_sb, op0=ALU.mult, op1=ALU.add)
                    nc.sync.dma_start(out=out_d[rows, :], in_=ores[:])

                def emit_a_epilogue():
                    if masked:
                        # masked tail keys 512..1023: weight-1
                        for j in range(4, 8):
                            nc.tensor.matmul(
                                oA[:], ones128[:], Vn[:, j, :],
                                start=False, stop=(j == 7))
                        nc.vector.tensor_scalar(
                            denA[:], rs[("A", 0)][:], float(S - WA),
                            None, ALU.add)
                    else:
                        nc.vector.tensor_add(
                            denA[:], rs[("A", 0)][:], rs[("A", 1)][:])
                    nc.vector.reciprocal(rcpA[:], denA[:])
                    emit_out("A", oA, rcpA, slice(0, 128))

                # all chunks except the final one get their full tails
                # inline; the final chunk's transposes/AV are emitted after
                # the A epilogue so its Z/exp have queue priority and the
                # A epilogue overlaps it
                for (tile_id, ci) in chunks[:-1]:
                    last = ci == (NCA if tile_id == "A" else NCB) - 1
                    emit_score(tile_id, ci)
                    emit_zexp(tile_id, ci, 1)
                    emit_trav(tile_id, ci, close=(last and not
                                                  (tile_id == "A" and masked)))
                emit_score("B", NCB - 1)
                emit_zexp("B", NCB - 1, 2)
                emit_a_epilogue()
                emit_trav("B", NCB - 1, close=True)
                nc.vector.tensor_add(
                    denB[:], rs[("B", 0)][:], rs[("B", 1)][:])
                nc.vector.reciprocal(rcpB[:], denB[:])
                emit_out("B", oB, rcpB, slice(128, 256))

    nc.finalize()
    return nc


_PROG_CACHE: dict[bool, bass.Bass] = {}


def _get_program(masked: bool) -> bass.Bass:
    if masked not in _PROG_CACHE:
        _PROG_CACHE[masked] = _build_program(masked)
    return _PROG_CACHE[masked]


def build_in_maps(x, Wq, bq, Wk, bk, Wv, bv, masked):
    wkt = np.ascontiguousarray(Wk.T.astype(np.float16))
    wqt = np.ascontiguousarray(Wq.T.astype(np.float16))
    wvt = np.ascontiguousarray(Wv.T.astype(np.float16))
    bcat = np.ascontiguousarray(
        np.concatenate(
            [bk.reshape(D, 1), bq.reshape(D, 1),
             np.tile(bv.reshape(1, D), (D, 1))], axis=1).astype(np.float32))
    WA = 512 if masked else 1024
    WB = 1024
    kidx = np.arange(S)
    in_maps = []
    for c in range(NCORES):
        b, l = divmod(c, 4)
        xb16 = x[b].astype(np.float16)
        xbt = np.ascontiguousarray(xb16.T)
        rows = np.concatenate(
            [128 * l + np.arange(128), 128 * (4 + l) + np.arange(128)])
        xqt = np.ascontiguousarray(xb16[rows].T)
        if masked:
            qa = (128 * l + np.arange(128))[:, None]
            qb = (128 * (4 + l) + np.arange(128))[:, None]
            m1a = (kidx[None, :WA] <= qa).astype(np.float16)
            m1b = (kidx[None, :WB] <= qb).astype(np.float16)
        else:
            m1a = np.ones((128, WA), np.float16)
            m1b = np.ones((128, WB), np.float16)
        in_maps.append({
            "xbt": xbt, "xqt": xqt, "wk": wkt, "wq": wqt, "wv": wvt,
            "bcat": bcat,
            "m1a": np.ascontiguousarray(m1a),
            "m1b": np.ascontiguousarray(m1b),
        })
    return in_maps


def assemble_out(results, masked):
    out = np.empty((B, S, D), dtype=np.float32)
    for c in range(NCORES):
        b, l = divmod(c, 4)
        res = results[c]["out"]
        out[b, 128 * l:128 * (l + 1)] = res[0:128]
        out[b, 128 * (4 + l):128 * (5 + l)] = res[128:256]
    return out


def kernel(x, Wq, bq, Wk, bk, Wv, bv, apply_causal_mask):
    x = np.ascontiguousarray(np.asarray(x, dtype=np.float32))
    Wq = np.asarray(Wq, dtype=np.float32)
    Wk = np.asarray(Wk, dtype=np.float32)
    Wv = np.asarray(Wv, dtype=np.float32)
    bq = np.asarray(bq, dtype=np.float32)
    bk = np.asarray(bk, dtype=np.float32)
    bv = np.asarray(bv, dtype=np.float32)
    masked = bool(int(np.asarray(apply_causal_mask)))

    nc = _get_program(masked)
    in_maps = build_in_maps(x, Wq, bq, Wk, bk, Wv, bv, masked)
    res = run_bass_kernel_spmd(nc, in_maps, list(range(NCORES))).results
    return assemble_out(res, masked)
